# revision 43
# baseline (speedup 1.0000x reference)
"""Multi-head latent attention (MLA) TRN2 kernel.

Sharding: batch(2) x query-sequence(4) over 8 cores. Each core:
  - runs the low-rank KV projection (kv_a + rmsnorm + rope rotation)
    for ONLY its own 512 tokens, then AllGathers the scaled latents and
    rope-paired keys across the 4 cores of its batch (replica groups
    [[0..3],[4..7]])
  - computes the Q path (q_a, rmsnorm, q_b, rope) for its 512 queries
  - kv_b + full attention for its 512 queries x 2048 keys x 16 heads
  - o_proj for its chunk -> output slice [512, 2048]
Host assembles the 8 slices into [B, T, HID].

Phase order hides all five collectives under compute:
  PE warmup (p-state ramp burn during the initial DMAs) ->
  kv_a latents (k-outer sweep streaming weight chunks) -> latent
  AllGather issued ~20us in -> kv_a rope + rotation + paired scatter ->
  FOUR slot-wise rope-key AllGathers (so the earliest kv-head pairs land
  before attention needs them) -> q_a -> q_b (rope heads first) ->
  kv_b preamble pipelined per head-pair, double-buffered, interleaved
  with attention -> attention -> o_proj.

Queue discipline matters in the cost model: the HWDGE descriptor rings
are shared, so a DMA that waits long (e.g. a gather land waiting on its
collective) poisons ring slots that later weight loads recycle through.
All collective-dependent lands therefore ride the SWDGE (gpsimd/Pool)
queue, placed in Pool program order so nothing time-critical queues
behind a long wait. Weight tiles stream on the ACT queue; xq + output
stores on the SP queue.

Matmul operands are bf16 except the decoupled-rope score slice: the
rotated rope halves of q and k are stored as fp8(e4m3) in a
[32 x 2 x tokens] layout so each rope score matmul runs as a single
fp8 DoubleRow matmul (two packed 32-row k-tiles, 0.5 cycles/row, the
full 64-dim rope contraction in half the cycles of a bf16 issue). PSUM
accumulation and the softmax statistics stay f32; only the rope slice
(1/3 of the score variance) sees fp8 rounding, measured 1.4e-2 max rel
err end to end.

The q rmsnorm scale is decoupled from the PE stream: ln*sqrt(rank) is
folded into the q_b rows on the host and the per-token rsqrt rides on
the PSUM->SBUF moves after q_b. Activations are feature-major
([feature, token]) so weight tiles act as lhsT directly; attention
computes scores transposed (s^T[k,q] = k^T q) so softmax needs no
transposes: exp on ACT, the denominator via an all-ones lhsT matmul
(two parallel bf16 accumulation chains per head — bf16 halves the DVE
cost that otherwise paces the attention inner loop), and P@V consumes
the transposed probabilities directly, pipelined one key-tile behind
the score stream. o_proj runs inside the attention pool scope (reusing
the score PSUM pool) so the last two heads' softmax finalizes hide
behind the first output tile's contraction.
"""

import math

import numpy as np

B, T, HID = 2, 2048, 2048
NH, NKV = 16, 8
NOPE, ROPE = 128, 64
HD = NOPE + ROPE  # 192
VD = 128
KV_RANK, Q_RANK = 512, 1536
EPS = 1e-6
THETA = 10000.0
NCORES = 8
TQ = B * T // NCORES  # 512 query tokens per core
P = 128
SCALE = 1.0 / math.sqrt(HD)

# Rope rows are stored "paired": each head's rotated rope halves (32+32
# rows) are stacked into one contiguous 64-row slot at base partition
# 64*(kvh%2), so the score-matmul lhsT(k)/rhs(q) base partitions match
# (PE only allows bases {0, 32, 64}).

_CACHE = {}


def _build_nc():
    import concourse.bass as bass  # noqa: F401
    import concourse.mybir as mybir
    from concourse import bacc
    from concourse.tile import TileContext

    F32 = mybir.dt.float32
    F32R = mybir.dt.float32r
    BF16 = mybir.dt.bfloat16
    F8 = mybir.dt.float8e4
    DR = mybir.MatmulPerfMode.DoubleRow
    AF = mybir.ActivationFunctionType
    ALU = mybir.AluOpType

    nc = bacc.Bacc(None, target_bir_lowering=False)

    xq_d = nc.dram_tensor("xq", [P, 16, TQ], BF16, kind="ExternalInput")
    qa_d = nc.dram_tensor("qa_w", [P, 12, 16, P], BF16, kind="ExternalInput")
    qb_d = nc.dram_tensor("qb_w", [P, 24, 12, P], BF16, kind="ExternalInput")
    kva_d = nc.dram_tensor("kva_w", [P, 16, 1024], BF16, kind="ExternalInput")
    kvb_d = nc.dram_tensor("kvb_w", [P, 4, 2048], BF16, kind="ExternalInput")
    o_d = nc.dram_tensor("o_w", [P, 4, 16, 512], BF16, kind="ExternalInput")
    cosq_d = nc.dram_tensor("cosq", [P, TQ], BF16, kind="ExternalInput")
    sinq_d = nc.dram_tensor("sinq", [P, TQ], BF16, kind="ExternalInput")
    onesb_d = nc.dram_tensor("ones_b", [P, P], BF16, kind="ExternalInput")
    # packed f32 tables: cols 0:128 all-ones (f32r lhsT for the softmax
    # denominator matmul), 128:132 kv ln weight * sqrt(rank), 132:134 eps
    tbl_d = nc.dram_tensor("tbl", [P, 134], F32R, kind="ExternalInput")
    qperm_d = nc.dram_tensor("qperm", [P, 4, P], BF16, kind="ExternalInput")
    # cross-core staging: this core's 512-key kv quarter + gathered full set
    kvl_p_d = nc.dram_tensor("kvl_p", [P, 4, TQ], BF16, kind="Internal")
    kprq_p_d = [nc.dram_tensor(f"kprq{j}_p", [64, 2, TQ], F8, kind="Internal")
                for j in range(4)]
    kvl_g_d = nc.dram_tensor("kvl_g", [4, P, 4, TQ], BF16, kind="Internal")
    kprg_d = [nc.dram_tensor(f"kprg{j}", [4, 64, 2, TQ], F8, kind="Internal")
              for j in range(4)]
    CC_GROUPS = [[0, 1, 2, 3], [4, 5, 6, 7]]
    out_d = nc.dram_tensor("out", [TQ, HID], F32, kind="ExternalOutput")

    with TileContext(nc) as tc:
        with tc.tile_pool(name="resident", bufs=1) as res:
            kv_latN = res.tile([P, 4, T], BF16, name="kv_latN")
            qnope = res.tile([P, NH, TQ], BF16, name="qnope")
            qrope = res.tile([64, 8, 2, TQ], F8, name="qrope")
            kpair = res.tile([64, 4, 2, T], F8, name="kpair")
            attn_sb = res.tile([P, NH, TQ], BF16, name="attn_sb")
            kvb_sb = res.tile([P, 4, 2048], BF16, name="kvb_sb")
            ones_sb = res.tile([P, P], BF16, name="ones_sb")
            tbl_sb = res.tile([P, 134], F32R, name="tbl_sb")

            # -- scat: rope-scatter sources, allocated at the TOP of SBUF
            # (side="right") so later phases' pools never overlap their
            # addresses and thus never wait on the background scatters.
            # -- pf1: kv_a/q inputs, freed before the attention phase.
            with (
                tc.tile_pool(name="scat", bufs=2, side="right") as scat,
                tc.tile_pool(name="pf1", bufs=1) as pf1,
            ):
                kva_sb = pf1.tile([P, 16, 1024], BF16, name="kva_sb")
                xq_sb = pf1.tile([P, 16, TQ], BF16, name="xq_sb")
                qperm_sb = pf1.tile([P, 4, P], BF16, name="qperm_sb")
                cosq_sb = scat.tile([P, TQ], BF16, name="cosq_sb", bufs=1)
                sinq_sb = scat.tile([P, TQ], BF16, name="sinq_sb", bufs=1)

                # ---- input streams ----
                # sync(SP) queue: xq chunks, then ONLY the collective-
                #   dependent gather lands + output stores (their sem waits
                #   hold the SP sequencer, which nothing else runs on)
                # scalar(ACT) queue: kva cols, qa/qb weight tiles, latent
                #   stage-out, odd kpr scatters, kvb, o_w
                for c in range(4):
                    nc.sync.dma_start(
                        xq_sb[:, 4 * c : 4 * c + 4, :],
                        xq_d[:, 4 * c : 4 * c + 4, :],
                    )
                    nc.scalar.dma_start(
                        kva_sb[:, 4 * c : 4 * c + 4, 0:512],
                        kva_d[:, 4 * c : 4 * c + 4, 0:512],
                    )
                for c in range(2):
                    nc.scalar.dma_start(
                        kva_sb[:, 8 * c : 8 * c + 8, 512:1024],
                        kva_d[:, 8 * c : 8 * c + 8, 512:1024],
                    )
                nc.scalar.dma_start(kvb_sb[:], kvb_d[:, :, :])
                wsrc0 = res.tile([P, 64], BF16, name="wsrc0")
                nc.vector.memset(wsrc0[:], 0.0)
                nc.gpsimd.dma_start(ones_sb[:], onesb_d[:, :])
                nc.gpsimd.dma_start(tbl_sb[:], tbl_d[:, :])
                nc.gpsimd.dma_start(cosq_sb[:], cosq_d[:, :])
                nc.gpsimd.dma_start(sinq_sb[:], sinq_d[:, :])
                nc.gpsimd.dma_start(qperm_sb[:], qperm_d[:, :, :])

                # ---- PE warmup: burn the p-state ramp during initial DMA.
                with (
                    tc.tile_pool(name="wu", bufs=1) as wu,
                    tc.tile_pool(name="wups", bufs=1, space="PSUM") as wups,
                ):
                    wps = wups.tile([P, 64], F32, tag="wu")
                    for _ in range(128):
                        nc.tensor.matmul(
                            wps[0:64, :], wsrc0[:, :], wsrc0[:, :],
                            start=True, stop=True,
                        )

                # ------------- P1: kv_a for THIS core's 512 tokens ---------
                # Latent sweep first (k-outer so matmuls consume weight
                # chunks as they land), stage + AllGather A. Then the rope
                # sweep, rotation, paired scatter, AllGather B.
                with (
                    tc.tile_pool(name="p1l", bufs=1) as p1l,
                    tc.tile_pool(name="p1s", bufs=2) as p1s,
                    tc.tile_pool(name="p1ps", bufs=1, space="PSUM") as p1ps,
                    tc.tile_pool(name="p1ps1", bufs=1, space="PSUM") as p1ps1,
                ):
                    kvl_loc = p1l.tile([P, 4, TQ], BF16, name="kvl_loc")
                    raw1 = p1l.tile([P, 2, TQ], BF16, name="raw1")
                    raw2 = p1l.tile([P, 2, TQ], BF16, name="raw2")

                    # latent sweep: 4 live psum accumulators (m 0..3), k-outer
                    # so matmuls consume kva weight chunks as they land
                    lps = {}
                    for m in range(4):
                        lps[m] = p1ps.tile([P, TQ], F32, tag=f"kl{m}",
                                           name=f"kl{m}")
                    for k in range(16):
                        for m in range(4):
                            nc.tensor.matmul(
                                lps[m][:],
                                kva_sb[:, k, m * P : (m + 1) * P],
                                xq_sb[:, k, :],
                                start=(k == 0), stop=(k == 15),
                            )
                    ksumsq = p1ps1.tile([P, TQ], F32, tag="ksumsq")
                    for m in range(4):
                        nc.scalar.copy(kvl_loc[:, m, :], lps[m][:])
                    for m in range(4):
                        sq = p1s.tile([P, TQ], BF16, tag="ksq")
                        nc.vector.tensor_tensor(
                            sq[:], kvl_loc[:, m, :], kvl_loc[:, m, :], ALU.mult
                        )
                        nc.tensor.matmul(
                            ksumsq[:], ones_sb[:], sq[:],
                            start=(m == 0), stop=(m == 3),
                        )
                    ksqt = p1s.tile([P, TQ], F32, tag="ksqt", bufs=1)
                    nc.scalar.activation(
                        ksqt[:], ksumsq[:], AF.Sqrt, bias=tbl_sb[:, 132:133]
                    )
                    krs = p1s.tile([P, TQ], F32, tag="krs", bufs=1)
                    nc.vector.reciprocal(krs[:], ksqt[:])
                    for m in range(4):
                        nc.vector.scalar_tensor_tensor(
                            kvl_loc[:, m, :], kvl_loc[:, m, :],
                            tbl_sb[:, 128 + m : 129 + m], krs[:],
                            ALU.mult, ALU.mult,
                        )
                    nc.scalar.dma_start(kvl_p_d[:, :, :], kvl_loc[:, :, :])
                    nc.gpsimd.collective_compute(
                        "AllGather", ALU.bypass, CC_GROUPS,
                        ins=[kvl_p_d[:, :, :]], outs=[kvl_g_d[:, :, :, :]],
                    )

                    # rope sweep (kva cols 512:1024 -> m 4..7), reuses the
                    # latent psum tags
                    rps = {}
                    for m in range(4):
                        tg = f"kl{m}" if m < 2 else f"kr{m}"
                        rps[m] = p1ps.tile([P, TQ], F32, tag=tg,
                                           name=f"kr{m}")
                    for m in range(4):
                        for k in range(16):
                            nc.tensor.matmul(
                                rps[m][:],
                                kva_sb[:, k, 512 + m * P : 512 + (m + 1) * P],
                                xq_sb[:, k, :],
                                start=(k == 0), stop=(k == 15),
                            )
                    for m in range(4):
                        dst = raw1 if m < 2 else raw2
                        nc.scalar.copy(dst[:, m % 2, :], rps[m][:])

                    # rotate own keys (the rope tables for them are the
                    # query tables) and scatter straight to the DRAM part
                    ckb = cosq_sb[:, None, :].to_broadcast((P, 2, TQ))
                    skb = sinq_sb[:, None, :].to_broadcast((P, 2, TQ))
                    rt = scat.tile([P, 2, TQ], BF16, tag="rtmp", bufs=2)
                    r1 = scat.tile([P, 2, TQ], F8, tag="krot1", bufs=1)
                    rc1 = scat.tile([P, 2, TQ], BF16, tag="rc1", bufs=1)
                    nc.vector.tensor_tensor(rt[:], raw2[:], skb, ALU.mult)
                    nc.vector.tensor_tensor(rc1[:], raw1[:], ckb, ALU.mult)
                    nc.vector.tensor_tensor(r1[:], rc1[:], rt[:], ALU.subtract)
                    rt2 = scat.tile([P, 2, TQ], BF16, tag="rtmp", bufs=2)
                    rc2 = scat.tile([P, 2, TQ], BF16, tag="rc2", bufs=1)
                    r2 = scat.tile([P, 2, TQ], F8, tag="krot2", bufs=1)
                    nc.vector.tensor_tensor(rt2[:], raw1[:], skb, ALU.mult)
                    nc.vector.tensor_tensor(rc2[:], raw2[:], ckb, ALU.mult)
                    nc.vector.tensor_tensor(r2[:], rc2[:], rt2[:], ALU.add)
                    # head kvh -> slot kvh//2, base 64*(kvh%2)
                    for kvh in range(NKV):
                        t_, i = kvh // 4, kvh % 4
                        bb = 32 * (kvh % 2)
                        eng = nc.gpsimd if kvh % 2 == 0 else nc.scalar
                        kprd = kprq_p_d[kvh // 2]
                        eng.dma_start(
                            kprd[bb : bb + 32, 0, :],
                            r1[i * 32 : (i + 1) * 32, t_, :],
                        )
                        eng.dma_start(
                            kprd[bb : bb + 32, 1, :],
                            r2[i * 32 : (i + 1) * 32, t_, :],
                        )
                    for j in range(4):
                        nc.gpsimd.collective_compute(
                            "AllGather", ALU.bypass, CC_GROUPS,
                            ins=[kprq_p_d[j][:, :, :]], outs=[kprg_d[j][:, :, :, :]],
                        )
                    # land the gathered latents on the SWDGE queue: its ring
                    # is private, so the wait on the collective cannot poison
                    # the HWDGE rings the weight streams recycle through
                    for r in range(4):
                        ksl = slice(r * TQ, (r + 1) * TQ)
                        nc.gpsimd.dma_start(
                            kv_latN[:, :, ksl], kvl_g_d[r, :, :, :]
                        )

                # ------------- P2: q path -------------
                with (
                    tc.tile_pool(name="p2", bufs=1) as p2,
                    tc.tile_pool(name="p2w", bufs=4) as p2w,
                    tc.tile_pool(name="p2s", bufs=2) as p2s,
                    tc.tile_pool(name="p2ps", bufs=2, space="PSUM") as p2ps,
                    tc.tile_pool(name="p2ps1", bufs=1, space="PSUM") as p2ps1,
                ):
                    q_lat = p2.tile([P, 12, TQ], BF16, name="q_lat")
                    rs_q = p2.tile([P, TQ], F32, name="rs_q")

                    # q_a + rmsnorm statistics (ln*rs applied after q_b:
                    # ln is folded into the q_b rows on the host, rs is a
                    # per-token scale that commutes with q_b)
                    sumsq = p2ps1.tile([P, TQ], F32, tag="qsumsq")
                    for m in range(12):
                        wt = p2w.tile([P, 16, P], BF16, tag="w", name="wt")
                        nc.scalar.dma_start(wt[:], qa_d[:, m, :, :])
                        ps = p2ps.tile([P, TQ], F32, tag="mm", name="ps")
                        for k in range(16):
                            nc.tensor.matmul(
                                ps[:], wt[:, k, :], xq_sb[:, k, :],
                                start=(k == 0), stop=(k == 15),
                            )
                        nc.vector.tensor_copy(q_lat[:, m, :], ps[:])
                        sq = p2s.tile([P, TQ], BF16, tag="sq", name="sq", bufs=1)
                        if m < 2:
                            nc.vector.tensor_tensor(
                                sq[:], q_lat[:, m, :], q_lat[:, m, :], ALU.mult
                            )
                        else:
                            nc.scalar.square(sq[:], ps[:])
                        nc.tensor.matmul(
                            sumsq[:], ones_sb[:], sq[:],
                            start=(m == 0), stop=(m == 11),
                        )
                    sqt = p2s.tile([P, TQ], F32, tag="sqt", bufs=1)
                    nc.scalar.activation(
                        sqt[:], sumsq[:], AF.Sqrt, bias=tbl_sb[:, 133:134]
                    )
                    nc.vector.reciprocal(rs_q[:], sqt[:])

                    # q_b: nope heads to qnope, rope raw kept for rotation;
                    # the rs_q normalization rides on the PSUM->SBUF move.
                    # Host orders the rope halves in rounds of 8 heads:
                    # m=16,17: q1(h0-7), m=18,19: q2(h0-7),
                    # m=20,21: q1(h8-15), m=22,23: q2(h8-15).
                    qraws = {}
                    for m in list(range(16, 24)) + list(range(16)):
                        wt = p2w.tile([P, 16, P], BF16, tag="w")
                        nc.scalar.dma_start(wt[:, :12, :], qb_d[:, m, :, :])
                        ps = p2ps.tile([P, TQ], F32, tag="mm")
                        for k in range(12):
                            nc.tensor.matmul(
                                ps[:], wt[:, k, :], q_lat[:, k, :],
                                start=(k == 0), stop=(k == 11),
                            )
                        if m < 16:
                            dst = qnope[:, m, :]
                        else:
                            j = m - 16
                            half, idx = (j // 2) % 2, j % 2
                            if idx == 0:
                                qraws[half] = scat.tile(
                                    [P, 2, TQ], BF16, tag=f"qraw{half}",
                                    bufs=2, name=f"qraw{half}",
                                )
                            dst = qraws[half][:, idx, :]
                        nc.vector.tensor_tensor(dst, ps[:], rs_q[:], ALU.mult)
                        if m >= 16 and m % 4 == 3:
                            # rotate this round's 8 heads and scatter to the
                            # paired layout via the GPSIMD (SWDGE) queue
                            rnd = (m - 16) // 4
                            cb = cosq_sb[:, None, :].to_broadcast((P, 2, TQ))
                            sb_ = sinq_sb[:, None, :].to_broadcast((P, 2, TQ))
                            qr1, qr2 = qraws[0], qraws[1]
                            qt = scat.tile([P, 2, TQ], BF16, tag="qrtmp", bufs=2)
                            qc1 = scat.tile([P, 2, TQ], BF16, tag="qc1", bufs=2)
                            qo1 = scat.tile([P, 2, TQ], F8, tag="qrot1")
                            qo2 = scat.tile([P, 2, TQ], F8, tag="qrot2")
                            nc.vector.tensor_tensor(qt[:], qr2[:], sb_, ALU.mult)
                            nc.vector.tensor_tensor(qc1[:], qr1[:], cb, ALU.mult)
                            nc.vector.tensor_tensor(qo1[:], qc1[:], qt[:], ALU.subtract)
                            qt2 = scat.tile([P, 2, TQ], BF16, tag="qrtmp", bufs=2)
                            qc2 = scat.tile([P, 2, TQ], BF16, tag="qc2", bufs=2)
                            nc.vector.tensor_tensor(qt2[:], qr1[:], sb_, ALU.mult)
                            nc.vector.tensor_tensor(qc2[:], qr2[:], cb, ALU.mult)
                            nc.vector.tensor_tensor(qo2[:], qc2[:], qt2[:], ALU.add)
                            # head qh -> slot 2*(qh//4)+qh%2, base 64*((qh//2)%2)
                            for qh in range(8 * rnd, 8 * rnd + 8):
                                slot = 2 * (qh // 4) + qh % 2
                                bb = 32 * ((qh // 2) % 2)
                                src_r = (qh % 4) * 32
                                src_t = (qh % 8) // 4
                                nc.gpsimd.dma_start(
                                    qrope[bb : bb + 32, slot, 0, :],
                                    qo1[src_r : src_r + 32, src_t, :],
                                )
                                nc.gpsimd.dma_start(
                                    qrope[bb : bb + 32, slot, 1, :],
                                    qo2[src_r : src_r + 32, src_t, :],
                                )

                    # land the gathered rope-key quarters (SWDGE queue,
                    # after the qrope scatters in Pool order; each slot only
                    # waits its own collective)
                    for j in range(4):
                        for r in range(4):
                            ksl = slice(r * TQ, (r + 1) * TQ)
                            nc.gpsimd.dma_start(
                                kpair[:, j, :, ksl], kprg_d[j][r, :, :, :]
                            )

            # ------------- P3 + P4 (pf1 SBUF freed) -----------------------
            with tc.tile_pool(name="oww", bufs=4) as oww:
                ow_tiles = {}

                def ow_load(n, eng):
                    ow = oww.tile([P, 16, 512], BF16, tag="ow")
                    eng.dma_start(ow[:], o_d[:, n, :, :])
                    ow_tiles[n] = ow

                with (
                    tc.tile_pool(name="p3", bufs=2) as p3,
                    tc.tile_pool(name="p3q", bufs=2) as p3q,
                    tc.tile_pool(name="p3p", bufs=4) as p3p,
                    tc.tile_pool(name="scps", bufs=4, space="PSUM") as scps,
                    tc.tile_pool(name="atps", bufs=2, space="PSUM") as atps,
                    tc.tile_pool(name="prps", bufs=2, space="PSUM") as prps,
                ):
                    pending = []

                    def finalize(item):
                        dsum, at, qh = item
                        dn = scps.tile([P, TQ], F32, tag="sc")
                        nc.tensor.matmul(
                            dn[:], ones_sb[:], dsum[:], start=True, stop=True
                        )
                        rec = p3q.tile([P, TQ], F32, tag="rec")
                        nc.vector.reciprocal(rec[:], dn[:])
                        nc.vector.tensor_tensor(
                            attn_sb[:, qh, :], at[:], rec[:], ALU.mult
                        )

                    # kv_b preamble for ONE head-pair: needs only the
                    # gathered LATENTS. Pipelined 2 stages ahead of the
                    # attention loop (double-buffered), so preambles hp0+hp1
                    # bridge the window between the latent gather and the
                    # rope-key gather while hp2/hp3 hide inside attention.
                    def preamble(hp):
                        kvh0 = 2 * hp
                        knp = p3.tile([P, 2, T], BF16, tag="knp", name="knp")
                        vp = p3.tile([P, 16, 256], BF16, tag="vp", name="vp")
                        for h2 in range(2):
                            wsl = slice((kvh0 + h2) * NOPE, (kvh0 + h2 + 1) * NOPE)
                            for n4 in range(4):
                                ksl = slice(n4 * 512, (n4 + 1) * 512)
                                ps = prps.tile([P, 512], F32, tag="pre")
                                for r in range(4):
                                    nc.tensor.matmul(
                                        ps[:], kvb_sb[:, r, wsl],
                                        kv_latN[:, r, ksl],
                                        start=(r == 0), stop=(r == 3),
                                    )
                                nc.vector.tensor_copy(knp[:, h2, ksl], ps[:])
                        vsl = slice(NKV * NOPE + kvh0 * VD, NKV * NOPE + (kvh0 + 2) * VD)
                        for kt in range(16):
                            ps = prps.tile([P, 512], F32, tag="pre")
                            for r in range(4):
                                nc.tensor.matmul(
                                    ps[:, :256],
                                    kv_latN[:, r, kt * P : (kt + 1) * P],
                                    kvb_sb[:, r, vsl],
                                    start=(r == 0), stop=(r == 3),
                                )
                            nc.scalar.copy(vp[:, kt, :], ps[:, :256])
                        if hp > 0:
                            # o_proj weight prefetch on the ACT queue
                            ow_load(hp - 1, nc.scalar)
                        if hp == 3:
                            ow_load(3, nc.scalar)
                        return knp, vp

                    kvp = {0: preamble(0), 1: preamble(1)}
                    for hp in range(4):  # kv-head pairs
                        kvh0 = 2 * hp
                        knp, vp = kvp.pop(hp)

                        for j4 in range(4):
                            qh = 4 * hp + j4
                            kvh = qh // 2
                            h2 = kvh - kvh0
                            b = 32 * (kvh % 2)
                            slot = 2 * (qh // 4) + qh % 2
                            dsum = p3q.tile([P, TQ], BF16, tag="dsum")
                            dsum2 = p3q.tile([P, TQ], BF16, tag="dsum2")
                            at = atps.tile([P, TQ], F32, tag="at")
                            pts = {}
                            for kt in range(16):
                                sc = scps.tile([P, TQ], F32, tag="sc")
                                nc.tensor.matmul(
                                    sc[:],
                                    knp[:, h2, kt * P : (kt + 1) * P],
                                    qnope[:, qh, :],
                                    start=True, stop=False,
                                )
                                nc.tensor.matmul(
                                    sc[:],
                                    kpair[b : b + 32, kvh // 2, :, kt * P : (kt + 1) * P],
                                    qrope[b : b + 32, slot, :, :],
                                    start=False, stop=True,
                                    perf_mode=DR,
                                )
                                pt = p3p.tile([P, TQ], BF16, tag="pt")
                                nc.scalar.activation(
                                    pt[:], sc[:], AF.Exp, scale=float(SCALE)
                                )
                                pts[kt] = pt
                                d_ = dsum if kt % 2 == 0 else dsum2
                                if kt < 2:
                                    nc.vector.tensor_copy(d_[:], pt[:])
                                else:
                                    nc.vector.tensor_tensor(
                                        d_[:], d_[:], pt[:], ALU.add
                                    )
                                if kt > 0:  # PV one stage behind scores
                                    nc.tensor.matmul(
                                        at[:],
                                        vp[:, kt - 1, h2 * VD : (h2 + 1) * VD],
                                        pts[kt - 1][:],
                                        start=(kt == 1), stop=False,
                                    )
                                    del pts[kt - 1]
                            nc.tensor.matmul(
                                at[:],
                                vp[:, 15, h2 * VD : (h2 + 1) * VD],
                                pts[15][:],
                                start=False, stop=True,
                            )
                            nc.vector.tensor_tensor(
                                dsum[:], dsum[:], dsum2[:], ALU.add
                            )
                            pending.append((dsum, at, qh))
                            if len(pending) == 2:
                                finalize(pending.pop(0))
                        if hp + 2 < 4:
                            kvp[hp + 2] = preamble(hp + 2)
                    # ---- P4: o_proj, inside the attention pools so the
                    # last two heads' finalizes hide behind the first chain
                    for n in range(4):
                        ow = ow_tiles[n]
                        for mt in range(4):
                            last = n == 3 and mt == 3
                            if not last:
                                ps = scps.tile([P, TQ], F32, tag="sc",
                                               name="ops")
                                for h in range(NH):
                                    if h == 14 and pending:
                                        while pending:
                                            finalize(pending.pop(0))
                                    nc.tensor.matmul(
                                        ps[:],
                                        attn_sb[:, h, mt * P : (mt + 1) * P],
                                        ow[:, h, :],
                                        start=(h == 0), stop=(h == 15),
                                    )
                                st = p3q.tile([P, TQ], F32, tag="st",
                                              name="st")
                                nc.scalar.copy(st[:], ps[:])
                                nc.sync.dma_start(
                                    out_d[mt * P : (mt + 1) * P,
                                          n * 512 : (n + 1) * 512],
                                    st[:],
                                )
                                continue
                            # split the final tile so the first half's copy
                            # and store overlap the second half's matmuls
                            for c0, c1 in ((0, 384), (384, 512)):
                                w = c1 - c0
                                ps = scps.tile([P, TQ], F32, tag="sc",
                                               name="ops2")
                                for h in range(NH):
                                    nc.tensor.matmul(
                                        ps[:, :w],
                                        attn_sb[:, h, mt * P : (mt + 1) * P],
                                        ow[:, h, c0:c1],
                                        start=(h == 0), stop=(h == 15),
                                    )
                                st = p3q.tile([P, TQ], F32, tag="st",
                                              name="st2")
                                nc.scalar.copy(st[:, :w], ps[:, :w])
                                nc.sync.dma_start(
                                    out_d[mt * P : (mt + 1) * P,
                                          n * 512 + c0 : n * 512 + c1],
                                    st[:, :w],
                                )

    nc.finalize()
    return nc


def _host_prep(inputs):
    import ml_dtypes

    BF = ml_dtypes.bfloat16

    def bf(a):
        return np.ascontiguousarray(np.asarray(a, dtype=np.float32).astype(BF))

    x = np.asarray(inputs["hidden_states"], dtype=np.float32)

    qa_w = np.asarray(inputs["q_a_w"], np.float32)  # [HID, Q_RANK]
    qa_t = bf(qa_w.reshape(16, P, 12, P).transpose(1, 2, 0, 3))

    # fold the q rmsnorm weight (and the sqrt(rank) factor of the mean)
    # into the q_b rows; the per-token rsqrt is applied after q_b on-device
    lnq = (np.asarray(inputs["q_a_ln_w"], np.float64) * math.sqrt(Q_RANK)).astype(
        np.float32
    )
    qb = np.asarray(inputs["q_b_w"], np.float32) * lnq[:, None]
    qb = qb.reshape(Q_RANK, NH, HD)
    nope_cols = qb[:, :, :NOPE].reshape(Q_RANK, NH * NOPE)
    rope1 = qb[:, :, NOPE : NOPE + 32].reshape(Q_RANK, NH * 32)
    rope2 = qb[:, :, NOPE + 32 :].reshape(Q_RANK, NH * 32)
    # rope halves in rounds of 8 heads: q1(h0-7), q2(h0-7), q1(h8-15), q2(h8-15)
    qb_cols = np.concatenate(
        [nope_cols, rope1[:, :256], rope2[:, :256], rope1[:, 256:], rope2[:, 256:]],
        axis=1,
    )  # [1536, 3072]
    qb_t = bf(qb_cols.reshape(12, P, 24, P).transpose(1, 2, 0, 3))

    kva = np.asarray(inputs["kv_a_w"], np.float32)
    lat = kva[:, :KV_RANK]
    krope = kva[:, KV_RANK:].reshape(HID, NKV, ROPE)
    kr1 = krope[:, :, :32].reshape(HID, NKV * 32)
    kr2 = krope[:, :, 32:].reshape(HID, NKV * 32)
    kva_cols = np.concatenate([lat, kr1, kr2], axis=1)  # [2048, 1024]
    kva_t = bf(kva_cols.reshape(16, P, 1024).transpose(1, 0, 2))

    kvb = np.asarray(inputs["kv_b_w"], np.float32).reshape(KV_RANK, NKV, NOPE + VD)
    knope_cols = kvb[:, :, :NOPE].reshape(KV_RANK, NKV * NOPE)
    v_cols = kvb[:, :, NOPE:].reshape(KV_RANK, NKV * VD)
    kvb_cols = np.concatenate([knope_cols, v_cols], axis=1)  # [512, 2048]
    kvb_t = bf(kvb_cols.reshape(4, P, 2048).transpose(1, 0, 2))

    o_w = np.asarray(inputs["o_w"], np.float32)  # [NH*VD, HID]
    o_t = bf(o_w.reshape(16, P, 4, 512).transpose(1, 2, 0, 3))

    lnkv = (
        (np.asarray(inputs["kv_a_ln_w"], np.float64) * math.sqrt(KV_RANK))
        .astype(np.float32)
        .reshape(4, P)
        .T
    )
    tbl = np.empty((P, 134), np.float32)
    tbl[:, 0:128] = 1.0
    tbl[:, 128:132] = lnkv
    tbl[:, 132] = EPS * KV_RANK
    tbl[:, 133] = EPS * Q_RANK

    inv_freq = 1.0 / (THETA ** (np.arange(0, ROPE, 2, dtype=np.float32) / ROPE))
    t = np.arange(T, dtype=np.float32)
    freqs = np.outer(t, inv_freq).astype(np.float32)
    cosk = np.tile(np.cos(freqs).T, (4, 1))  # [128, T]
    sink = np.tile(np.sin(freqs).T, (4, 1))
    cosk_b, sink_b = bf(cosk), bf(sink)
    ones_b = np.ones((P, P), BF)
    qperm = np.zeros((4, P, P), np.float32)
    for q in list(range(0, 32)) + list(range(64, 96)):
        qperm[0, q, q] = 1.0
        qperm[1, q, q + 32] = 1.0
    for q in list(range(32, 64)) + list(range(96, 128)):
        qperm[2, q, q - 32] = 1.0
        qperm[3, q, q] = 1.0
    qperm_t = bf(qperm.transpose(1, 0, 2))

    in_maps = []
    for c in range(NCORES):
        b, qc = c // 4, c % 4
        xTb = x[b].T  # [HID, T]
        qoff = qc * TQ
        xq_t = bf(xTb[:, qoff : qoff + TQ].reshape(16, P, TQ).transpose(1, 0, 2))
        in_maps.append(
            {
                "xq": xq_t,
                "qa_w": qa_t,
                "qb_w": qb_t,
                "kva_w": kva_t,
                "kvb_w": kvb_t,
                "o_w": o_t,
                "cosq": np.ascontiguousarray(cosk_b[:, qoff : qoff + TQ]),
                "sinq": np.ascontiguousarray(sink_b[:, qoff : qoff + TQ]),
                "ones_b": ones_b,
                "tbl": tbl,
                "qperm": qperm_t,
            }
        )
    return in_maps


def get_nc():
    if "nc" not in _CACHE:
        _CACHE["nc"] = _build_nc()
    return _CACHE["nc"]


def kernel(**inputs) -> np.ndarray:
    from concourse.bass_utils import run_bass_kernel_spmd

    nc = get_nc()
    in_maps = _host_prep(inputs)
    res = run_bass_kernel_spmd(nc, in_maps, core_ids=list(range(NCORES)))
    _CACHE["last_result"] = res
    outs = [res.results[c]["out"] for c in range(NCORES)]
    full = np.stack(
        [np.concatenate([outs[b * 4 + qc] for qc in range(4)], axis=0) for b in range(B)]
    )
    return full.astype(np.float32)


# revision 48
# speedup vs baseline: 1.0013x; 1.0013x over previous
"""Multi-head latent attention (MLA) TRN2 kernel.

Sharding: batch(2) x query-sequence(4) over 8 cores. Each core:
  - runs the low-rank KV projection (kv_a + rmsnorm + rope rotation)
    for ONLY its own 512 tokens, then AllGathers the scaled latents and
    rope-paired keys across the 4 cores of its batch (replica groups
    [[0..3],[4..7]])
  - computes the Q path (q_a, rmsnorm, q_b, rope) for its 512 queries
  - kv_b + full attention for its 512 queries x 2048 keys x 16 heads
  - o_proj for its chunk -> output slice [512, 2048]
Host assembles the 8 slices into [B, T, HID].

Phase order hides all five collectives under compute:
  PE warmup (p-state ramp burn during the initial DMAs) ->
  kv_a latents (k-outer sweep streaming weight chunks) -> latent
  AllGather issued ~20us in -> kv_a rope + rotation + paired scatter ->
  FOUR slot-wise rope-key AllGathers (so the earliest kv-head pairs land
  before attention needs them) -> q_a -> q_b (rope heads first) ->
  kv_b preamble pipelined per head-pair, double-buffered, interleaved
  with attention -> attention -> o_proj.

Queue discipline matters in the cost model: the HWDGE descriptor rings
are shared, so a DMA that waits long (e.g. a gather land waiting on its
collective) poisons ring slots that later weight loads recycle through.
All collective-dependent lands therefore ride the SWDGE (gpsimd/Pool)
queue, placed in Pool program order so nothing time-critical queues
behind a long wait. Weight tiles stream on the ACT queue; xq + output
stores on the SP queue.

Matmul operands are bf16 except the decoupled-rope score slice: the
rotated rope halves of q and k are stored as fp8(e4m3) in a
[32 x 2 x tokens] layout so each rope score matmul runs as a single
fp8 DoubleRow matmul (two packed 32-row k-tiles, 0.5 cycles/row, the
full 64-dim rope contraction in half the cycles of a bf16 issue). PSUM
accumulation and the softmax statistics stay f32; only the rope slice
(1/3 of the score variance) sees fp8 rounding, measured 1.4e-2 max rel
err end to end.

The q rmsnorm scale is decoupled from the PE stream: ln*sqrt(rank) is
folded into the q_b rows on the host and the per-token rsqrt rides on
the PSUM->SBUF moves after q_b. Activations are feature-major
([feature, token]) so weight tiles act as lhsT directly; attention
computes scores transposed (s^T[k,q] = k^T q) so softmax needs no
transposes: exp on ACT, the denominator via an all-ones lhsT matmul
(two parallel bf16 accumulation chains per head — bf16 halves the DVE
cost that otherwise paces the attention inner loop), and P@V consumes
the transposed probabilities directly, pipelined one key-tile behind
the score stream. o_proj runs inside the attention pool scope (reusing
the score PSUM pool) so the last two heads' softmax finalizes hide
behind the first output tile's contraction.
"""

import math

import numpy as np

B, T, HID = 2, 2048, 2048
NH, NKV = 16, 8
NOPE, ROPE = 128, 64
HD = NOPE + ROPE  # 192
VD = 128
KV_RANK, Q_RANK = 512, 1536
EPS = 1e-6
THETA = 10000.0
NCORES = 8
TQ = B * T // NCORES  # 512 query tokens per core
P = 128
SCALE = 1.0 / math.sqrt(HD)

# Rope rows are stored "paired": each head's rotated rope halves (32+32
# rows) are stacked into one contiguous 64-row slot at base partition
# 64*(kvh%2), so the score-matmul lhsT(k)/rhs(q) base partitions match
# (PE only allows bases {0, 32, 64}).

_CACHE = {}


def _build_nc():
    import concourse.bass as bass  # noqa: F401
    import concourse.mybir as mybir
    from concourse import bacc
    from concourse.tile import TileContext

    F32 = mybir.dt.float32
    F32R = mybir.dt.float32r
    BF16 = mybir.dt.bfloat16
    F8 = mybir.dt.float8e4
    DR = mybir.MatmulPerfMode.DoubleRow
    AF = mybir.ActivationFunctionType
    ALU = mybir.AluOpType

    nc = bacc.Bacc(None, target_bir_lowering=False)

    xq_d = nc.dram_tensor("xq", [P, 16, TQ], BF16, kind="ExternalInput")
    qa_d = nc.dram_tensor("qa_w", [P, 12, 16, P], BF16, kind="ExternalInput")
    qb_d = nc.dram_tensor("qb_w", [P, 24, 12, P], BF16, kind="ExternalInput")
    kva_d = nc.dram_tensor("kva_w", [P, 16, 1024], BF16, kind="ExternalInput")
    kvb_d = nc.dram_tensor("kvb_w", [P, 4, 2048], BF16, kind="ExternalInput")
    o_d = nc.dram_tensor("o_w", [P, 4, 16, 512], BF16, kind="ExternalInput")
    cosq_d = nc.dram_tensor("cosq", [P, TQ], BF16, kind="ExternalInput")
    sinq_d = nc.dram_tensor("sinq", [P, TQ], BF16, kind="ExternalInput")
    onesb_d = nc.dram_tensor("ones_b", [P, P], BF16, kind="ExternalInput")
    # packed f32 tables: cols 0:128 all-ones (f32r lhsT for the softmax
    # denominator matmul), 128:132 kv ln weight * sqrt(rank), 132:134 eps
    tbl_d = nc.dram_tensor("tbl", [P, 134], F32R, kind="ExternalInput")
    qperm_d = nc.dram_tensor("qperm", [P, 4, P], BF16, kind="ExternalInput")
    # cross-core staging: this core's 512-key kv quarter + gathered full set
    kvl_p_d = nc.dram_tensor("kvl_p", [P, 4, TQ], BF16, kind="Internal")
    kprq_p_d = [nc.dram_tensor(f"kprq{j}_p", [64, 2, TQ], F8, kind="Internal")
                for j in range(4)]
    kvl_g_d = nc.dram_tensor("kvl_g", [4, P, 4, TQ], BF16, kind="Internal")
    kprg_d = [nc.dram_tensor(f"kprg{j}", [4, 64, 2, TQ], F8, kind="Internal")
              for j in range(4)]
    CC_GROUPS = [[0, 1, 2, 3], [4, 5, 6, 7]]
    out_d = nc.dram_tensor("out", [TQ, HID], F32, kind="ExternalOutput")

    with TileContext(nc) as tc:
        with tc.tile_pool(name="resident", bufs=1) as res:
            kv_latN = res.tile([P, 4, T], BF16, name="kv_latN")
            qnope = res.tile([P, NH, TQ], BF16, name="qnope")
            qrope = res.tile([64, 8, 2, TQ], F8, name="qrope")
            kpair = res.tile([64, 4, 2, T], F8, name="kpair")
            attn_sb = res.tile([P, NH, TQ], BF16, name="attn_sb")
            kvb_sb = res.tile([P, 4, 2048], BF16, name="kvb_sb")
            ones_sb = res.tile([P, P], BF16, name="ones_sb")
            tbl_sb = res.tile([P, 134], F32R, name="tbl_sb")

            # -- scat: rope-scatter sources, allocated at the TOP of SBUF
            # (side="right") so later phases' pools never overlap their
            # addresses and thus never wait on the background scatters.
            # -- pf1: kv_a/q inputs, freed before the attention phase.
            with (
                tc.tile_pool(name="scat", bufs=2, side="right") as scat,
                tc.tile_pool(name="pf1", bufs=1) as pf1,
            ):
                kva_sb = pf1.tile([P, 16, 1024], BF16, name="kva_sb")
                xq_sb = pf1.tile([P, 16, TQ], BF16, name="xq_sb")
                qperm_sb = pf1.tile([P, 4, P], BF16, name="qperm_sb")
                cosq_sb = scat.tile([P, TQ], BF16, name="cosq_sb", bufs=1)
                sinq_sb = scat.tile([P, TQ], BF16, name="sinq_sb", bufs=1)

                # ---- input streams ----
                # sync(SP) queue: xq chunks, then ONLY the collective-
                #   dependent gather lands + output stores (their sem waits
                #   hold the SP sequencer, which nothing else runs on)
                # scalar(ACT) queue: kva cols, qa/qb weight tiles, latent
                #   stage-out, odd kpr scatters, kvb, o_w
                for c in range(4):
                    nc.sync.dma_start(
                        xq_sb[:, 4 * c : 4 * c + 4, :],
                        xq_d[:, 4 * c : 4 * c + 4, :],
                    )
                    nc.scalar.dma_start(
                        kva_sb[:, 4 * c : 4 * c + 4, 0:512],
                        kva_d[:, 4 * c : 4 * c + 4, 0:512],
                    )
                for c in range(2):
                    nc.scalar.dma_start(
                        kva_sb[:, 8 * c : 8 * c + 8, 512:1024],
                        kva_d[:, 8 * c : 8 * c + 8, 512:1024],
                    )
                nc.scalar.dma_start(kvb_sb[:], kvb_d[:, :, :])
                wsrc0 = res.tile([P, 64], BF16, name="wsrc0")
                nc.vector.memset(wsrc0[:], 0.0)
                nc.gpsimd.dma_start(ones_sb[:], onesb_d[:, :])
                nc.gpsimd.dma_start(tbl_sb[:], tbl_d[:, :])
                nc.gpsimd.dma_start(cosq_sb[:], cosq_d[:, :])
                nc.gpsimd.dma_start(sinq_sb[:], sinq_d[:, :])
                nc.gpsimd.dma_start(qperm_sb[:], qperm_d[:, :, :])

                # ---- PE warmup: burn the p-state ramp during initial DMA.
                with (
                    tc.tile_pool(name="wu", bufs=1) as wu,
                    tc.tile_pool(name="wups", bufs=1, space="PSUM") as wups,
                ):
                    wps = wups.tile([P, 64], F32, tag="wu")
                    for _ in range(96):
                        nc.tensor.matmul(
                            wps[0:64, :], wsrc0[:, :], wsrc0[:, :],
                            start=True, stop=True,
                        )

                # ------------- P1: kv_a for THIS core's 512 tokens ---------
                # Latent sweep first (k-outer so matmuls consume weight
                # chunks as they land), stage + AllGather A. Then the rope
                # sweep, rotation, paired scatter, AllGather B.
                with (
                    tc.tile_pool(name="p1l", bufs=1) as p1l,
                    tc.tile_pool(name="p1s", bufs=2) as p1s,
                    tc.tile_pool(name="p1ps", bufs=1, space="PSUM") as p1ps,
                    tc.tile_pool(name="p1ps1", bufs=1, space="PSUM") as p1ps1,
                ):
                    kvl_loc = p1l.tile([P, 4, TQ], BF16, name="kvl_loc")
                    raw1 = p1l.tile([P, 2, TQ], BF16, name="raw1")
                    raw2 = p1l.tile([P, 2, TQ], BF16, name="raw2")

                    # latent sweep: 4 live psum accumulators (m 0..3), k-outer
                    # so matmuls consume kva weight chunks as they land
                    lps = {}
                    for m in range(4):
                        lps[m] = p1ps.tile([P, TQ], F32, tag=f"kl{m}",
                                           name=f"kl{m}")
                    for k in range(16):
                        for m in range(4):
                            nc.tensor.matmul(
                                lps[m][:],
                                kva_sb[:, k, m * P : (m + 1) * P],
                                xq_sb[:, k, :],
                                start=(k == 0), stop=(k == 15),
                            )
                    ksumsq = p1ps1.tile([P, TQ], F32, tag="ksumsq")
                    for m in range(4):
                        nc.scalar.copy(kvl_loc[:, m, :], lps[m][:])
                    for m in range(4):
                        sq = p1s.tile([P, TQ], BF16, tag="ksq")
                        nc.vector.tensor_tensor(
                            sq[:], kvl_loc[:, m, :], kvl_loc[:, m, :], ALU.mult
                        )
                        nc.tensor.matmul(
                            ksumsq[:], ones_sb[:], sq[:],
                            start=(m == 0), stop=(m == 3),
                        )
                    ksqt = p1s.tile([P, TQ], F32, tag="ksqt", bufs=1)
                    nc.scalar.activation(
                        ksqt[:], ksumsq[:], AF.Sqrt, bias=tbl_sb[:, 132:133]
                    )
                    krs = p1s.tile([P, TQ], F32, tag="krs", bufs=1)
                    nc.vector.reciprocal(krs[:], ksqt[:])
                    for m in range(4):
                        nc.vector.scalar_tensor_tensor(
                            kvl_loc[:, m, :], kvl_loc[:, m, :],
                            tbl_sb[:, 128 + m : 129 + m], krs[:],
                            ALU.mult, ALU.mult,
                        )
                    nc.scalar.dma_start(kvl_p_d[:, :, :], kvl_loc[:, :, :])
                    nc.gpsimd.collective_compute(
                        "AllGather", ALU.bypass, CC_GROUPS,
                        ins=[kvl_p_d[:, :, :]], outs=[kvl_g_d[:, :, :, :]],
                    )

                    # rope sweep (kva cols 512:1024 -> m 4..7), reuses the
                    # latent psum tags
                    rps = {}
                    for m in range(4):
                        tg = f"kl{m}" if m < 2 else f"kr{m}"
                        rps[m] = p1ps.tile([P, TQ], F32, tag=tg,
                                           name=f"kr{m}")
                    for m in range(4):
                        for k in range(16):
                            nc.tensor.matmul(
                                rps[m][:],
                                kva_sb[:, k, 512 + m * P : 512 + (m + 1) * P],
                                xq_sb[:, k, :],
                                start=(k == 0), stop=(k == 15),
                            )
                    for m in range(4):
                        dst = raw1 if m < 2 else raw2
                        nc.scalar.copy(dst[:, m % 2, :], rps[m][:])

                    # rotate own keys (the rope tables for them are the
                    # query tables) and scatter straight to the DRAM part
                    ckb = cosq_sb[:, None, :].to_broadcast((P, 2, TQ))
                    skb = sinq_sb[:, None, :].to_broadcast((P, 2, TQ))
                    rt = scat.tile([P, 2, TQ], BF16, tag="rtmp", bufs=2)
                    r1 = scat.tile([P, 2, TQ], F8, tag="krot1", bufs=1)
                    rc1 = scat.tile([P, 2, TQ], BF16, tag="rc1", bufs=1)
                    nc.vector.tensor_tensor(rt[:], raw2[:], skb, ALU.mult)
                    nc.vector.tensor_tensor(rc1[:], raw1[:], ckb, ALU.mult)
                    nc.vector.tensor_tensor(r1[:], rc1[:], rt[:], ALU.subtract)
                    rt2 = scat.tile([P, 2, TQ], BF16, tag="rtmp", bufs=2)
                    rc2 = scat.tile([P, 2, TQ], BF16, tag="rc2", bufs=1)
                    r2 = scat.tile([P, 2, TQ], F8, tag="krot2", bufs=1)
                    nc.vector.tensor_tensor(rt2[:], raw1[:], skb, ALU.mult)
                    nc.vector.tensor_tensor(rc2[:], raw2[:], ckb, ALU.mult)
                    nc.vector.tensor_tensor(r2[:], rc2[:], rt2[:], ALU.add)
                    # head kvh -> slot kvh//2, base 64*(kvh%2)
                    for kvh in range(NKV):
                        t_, i = kvh // 4, kvh % 4
                        bb = 32 * (kvh % 2)
                        eng = nc.gpsimd if kvh % 2 == 0 else nc.scalar
                        kprd = kprq_p_d[kvh // 2]
                        eng.dma_start(
                            kprd[bb : bb + 32, 0, :],
                            r1[i * 32 : (i + 1) * 32, t_, :],
                        )
                        eng.dma_start(
                            kprd[bb : bb + 32, 1, :],
                            r2[i * 32 : (i + 1) * 32, t_, :],
                        )
                    for j in range(4):
                        nc.gpsimd.collective_compute(
                            "AllGather", ALU.bypass, CC_GROUPS,
                            ins=[kprq_p_d[j][:, :, :]], outs=[kprg_d[j][:, :, :, :]],
                        )
                    # land the gathered latents on the SWDGE queue: its ring
                    # is private, so the wait on the collective cannot poison
                    # the HWDGE rings the weight streams recycle through
                    for r in range(4):
                        ksl = slice(r * TQ, (r + 1) * TQ)
                        nc.gpsimd.dma_start(
                            kv_latN[:, :, ksl], kvl_g_d[r, :, :, :]
                        )

                # ------------- P2: q path -------------
                with (
                    tc.tile_pool(name="p2", bufs=1) as p2,
                    tc.tile_pool(name="p2w", bufs=4) as p2w,
                    tc.tile_pool(name="p2s", bufs=2) as p2s,
                    tc.tile_pool(name="p2ps", bufs=2, space="PSUM") as p2ps,
                    tc.tile_pool(name="p2ps1", bufs=1, space="PSUM") as p2ps1,
                ):
                    q_lat = p2.tile([P, 12, TQ], BF16, name="q_lat")
                    rs_q = p2.tile([P, TQ], F32, name="rs_q")

                    # q_a + rmsnorm statistics (ln*rs applied after q_b:
                    # ln is folded into the q_b rows on the host, rs is a
                    # per-token scale that commutes with q_b)
                    sumsq = p2ps1.tile([P, TQ], F32, tag="qsumsq")
                    for m in range(12):
                        wt = p2w.tile([P, 16, P], BF16, tag="w", name="wt")
                        nc.scalar.dma_start(wt[:], qa_d[:, m, :, :])
                        ps = p2ps.tile([P, TQ], F32, tag="mm", name="ps")
                        for k in range(16):
                            nc.tensor.matmul(
                                ps[:], wt[:, k, :], xq_sb[:, k, :],
                                start=(k == 0), stop=(k == 15),
                            )
                        nc.vector.tensor_copy(q_lat[:, m, :], ps[:])
                        sq = p2s.tile([P, TQ], BF16, tag="sq", name="sq", bufs=1)
                        if m < 2:
                            nc.vector.tensor_tensor(
                                sq[:], q_lat[:, m, :], q_lat[:, m, :], ALU.mult
                            )
                        else:
                            nc.scalar.square(sq[:], ps[:])
                        nc.tensor.matmul(
                            sumsq[:], ones_sb[:], sq[:],
                            start=(m == 0), stop=(m == 11),
                        )
                    sqt = p2s.tile([P, TQ], F32, tag="sqt", bufs=1)
                    nc.scalar.activation(
                        sqt[:], sumsq[:], AF.Sqrt, bias=tbl_sb[:, 133:134]
                    )
                    nc.vector.reciprocal(rs_q[:], sqt[:])

                    # q_b: nope heads to qnope, rope raw kept for rotation;
                    # the rs_q normalization rides on the PSUM->SBUF move.
                    # Host orders the rope halves in rounds of 8 heads:
                    # m=16,17: q1(h0-7), m=18,19: q2(h0-7),
                    # m=20,21: q1(h8-15), m=22,23: q2(h8-15).
                    qraws = {}
                    for m in list(range(16, 24)) + list(range(16)):
                        wt = p2w.tile([P, 16, P], BF16, tag="w")
                        nc.scalar.dma_start(wt[:, :12, :], qb_d[:, m, :, :])
                        ps = p2ps.tile([P, TQ], F32, tag="mm")
                        for k in range(12):
                            nc.tensor.matmul(
                                ps[:], wt[:, k, :], q_lat[:, k, :],
                                start=(k == 0), stop=(k == 11),
                            )
                        if m < 16:
                            dst = qnope[:, m, :]
                        else:
                            j = m - 16
                            half, idx = (j // 2) % 2, j % 2
                            if idx == 0:
                                qraws[half] = scat.tile(
                                    [P, 2, TQ], BF16, tag=f"qraw{half}",
                                    bufs=2, name=f"qraw{half}",
                                )
                            dst = qraws[half][:, idx, :]
                        nc.vector.tensor_tensor(dst, ps[:], rs_q[:], ALU.mult)
                        if m >= 16 and m % 4 == 3:
                            # rotate this round's 8 heads and scatter to the
                            # paired layout via the GPSIMD (SWDGE) queue
                            rnd = (m - 16) // 4
                            cb = cosq_sb[:, None, :].to_broadcast((P, 2, TQ))
                            sb_ = sinq_sb[:, None, :].to_broadcast((P, 2, TQ))
                            qr1, qr2 = qraws[0], qraws[1]
                            qt = scat.tile([P, 2, TQ], BF16, tag="qrtmp", bufs=2)
                            qc1 = scat.tile([P, 2, TQ], BF16, tag="qc1", bufs=2)
                            qo1 = scat.tile([P, 2, TQ], F8, tag="qrot1")
                            qo2 = scat.tile([P, 2, TQ], F8, tag="qrot2")
                            nc.vector.tensor_tensor(qt[:], qr2[:], sb_, ALU.mult)
                            nc.vector.tensor_tensor(qc1[:], qr1[:], cb, ALU.mult)
                            nc.vector.tensor_tensor(qo1[:], qc1[:], qt[:], ALU.subtract)
                            qt2 = scat.tile([P, 2, TQ], BF16, tag="qrtmp", bufs=2)
                            qc2 = scat.tile([P, 2, TQ], BF16, tag="qc2", bufs=2)
                            nc.vector.tensor_tensor(qt2[:], qr1[:], sb_, ALU.mult)
                            nc.vector.tensor_tensor(qc2[:], qr2[:], cb, ALU.mult)
                            nc.vector.tensor_tensor(qo2[:], qc2[:], qt2[:], ALU.add)
                            # head qh -> slot 2*(qh//4)+qh%2, base 64*((qh//2)%2)
                            for qh in range(8 * rnd, 8 * rnd + 8):
                                slot = 2 * (qh // 4) + qh % 2
                                bb = 32 * ((qh // 2) % 2)
                                src_r = (qh % 4) * 32
                                src_t = (qh % 8) // 4
                                nc.gpsimd.dma_start(
                                    qrope[bb : bb + 32, slot, 0, :],
                                    qo1[src_r : src_r + 32, src_t, :],
                                )
                                nc.gpsimd.dma_start(
                                    qrope[bb : bb + 32, slot, 1, :],
                                    qo2[src_r : src_r + 32, src_t, :],
                                )

                    # land the gathered rope-key quarters (SWDGE queue,
                    # after the qrope scatters in Pool order; each slot only
                    # waits its own collective)
                    for j in range(4):
                        for r in range(4):
                            ksl = slice(r * TQ, (r + 1) * TQ)
                            nc.gpsimd.dma_start(
                                kpair[:, j, :, ksl], kprg_d[j][r, :, :, :]
                            )

            # ------------- P3 + P4 (pf1 SBUF freed) -----------------------
            with tc.tile_pool(name="oww", bufs=4) as oww:
                ow_tiles = {}

                def ow_load(n, eng):
                    ow = oww.tile([P, 16, 512], BF16, tag="ow")
                    eng.dma_start(ow[:], o_d[:, n, :, :])
                    ow_tiles[n] = ow

                with (
                    tc.tile_pool(name="p3", bufs=2) as p3,
                    tc.tile_pool(name="p3q", bufs=2) as p3q,
                    tc.tile_pool(name="p3p", bufs=4) as p3p,
                    tc.tile_pool(name="scps", bufs=4, space="PSUM") as scps,
                    tc.tile_pool(name="atps", bufs=2, space="PSUM") as atps,
                    tc.tile_pool(name="prps", bufs=2, space="PSUM") as prps,
                ):
                    pending = []

                    def finalize(item):
                        dsum, at, qh = item
                        dn = scps.tile([P, TQ], F32, tag="sc")
                        nc.tensor.matmul(
                            dn[:], ones_sb[:], dsum[:], start=True, stop=True
                        )
                        rec = p3q.tile([P, TQ], F32, tag="rec")
                        nc.vector.reciprocal(rec[:], dn[:])
                        nc.vector.tensor_tensor(
                            attn_sb[:, qh, :], at[:], rec[:], ALU.mult
                        )

                    # kv_b preamble for ONE head-pair: needs only the
                    # gathered LATENTS. Pipelined 2 stages ahead of the
                    # attention loop (double-buffered), so preambles hp0+hp1
                    # bridge the window between the latent gather and the
                    # rope-key gather while hp2/hp3 hide inside attention.
                    def preamble(hp):
                        kvh0 = 2 * hp
                        knp = p3.tile([P, 2, T], BF16, tag="knp", name="knp")
                        vp = p3.tile([P, 16, 256], BF16, tag="vp", name="vp")
                        for h2 in range(2):
                            wsl = slice((kvh0 + h2) * NOPE, (kvh0 + h2 + 1) * NOPE)
                            for n4 in range(4):
                                ksl = slice(n4 * 512, (n4 + 1) * 512)
                                ps = prps.tile([P, 512], F32, tag="pre")
                                for r in range(4):
                                    nc.tensor.matmul(
                                        ps[:], kvb_sb[:, r, wsl],
                                        kv_latN[:, r, ksl],
                                        start=(r == 0), stop=(r == 3),
                                    )
                                nc.vector.tensor_copy(knp[:, h2, ksl], ps[:])
                        vsl = slice(NKV * NOPE + kvh0 * VD, NKV * NOPE + (kvh0 + 2) * VD)
                        for kt in range(16):
                            ps = prps.tile([P, 512], F32, tag="pre")
                            for r in range(4):
                                nc.tensor.matmul(
                                    ps[:, :256],
                                    kv_latN[:, r, kt * P : (kt + 1) * P],
                                    kvb_sb[:, r, vsl],
                                    start=(r == 0), stop=(r == 3),
                                )
                            if (hp == 1 and kt >= 12) or hp >= 2:
                                nc.vector.tensor_copy(vp[:, kt, :], ps[:, :256])
                            else:
                                nc.scalar.copy(vp[:, kt, :], ps[:, :256])
                        if hp > 0:
                            # o_proj weight prefetch on the ACT queue
                            ow_load(hp - 1, nc.scalar)
                        if hp == 3:
                            ow_load(3, nc.scalar)
                        return knp, vp

                    kvp = {0: preamble(0), 1: preamble(1)}
                    for hp in range(4):  # kv-head pairs
                        kvh0 = 2 * hp
                        knp, vp = kvp.pop(hp)

                        for j4 in range(4):
                            qh = 4 * hp + j4
                            kvh = qh // 2
                            h2 = kvh - kvh0
                            b = 32 * (kvh % 2)
                            slot = 2 * (qh // 4) + qh % 2
                            dsum = p3q.tile([P, TQ], BF16, tag="dsum")
                            dsum2 = p3q.tile([P, TQ], BF16, tag="dsum2")
                            at = atps.tile([P, TQ], F32, tag="at")
                            pts = {}
                            for kt in range(16):
                                sc = scps.tile([P, TQ], F32, tag="sc")
                                nc.tensor.matmul(
                                    sc[:],
                                    knp[:, h2, kt * P : (kt + 1) * P],
                                    qnope[:, qh, :],
                                    start=True, stop=False,
                                )
                                nc.tensor.matmul(
                                    sc[:],
                                    kpair[b : b + 32, kvh // 2, :, kt * P : (kt + 1) * P],
                                    qrope[b : b + 32, slot, :, :],
                                    start=False, stop=True,
                                    perf_mode=DR,
                                )
                                pt = p3p.tile([P, TQ], BF16, tag="pt")
                                nc.scalar.activation(
                                    pt[:], sc[:], AF.Exp, scale=float(SCALE)
                                )
                                pts[kt] = pt
                                d_ = dsum if kt % 2 == 0 else dsum2
                                if kt < 2:
                                    nc.vector.tensor_copy(d_[:], pt[:])
                                else:
                                    nc.vector.tensor_tensor(
                                        d_[:], d_[:], pt[:], ALU.add
                                    )
                                if kt > 0:  # PV one stage behind scores
                                    nc.tensor.matmul(
                                        at[:],
                                        vp[:, kt - 1, h2 * VD : (h2 + 1) * VD],
                                        pts[kt - 1][:],
                                        start=(kt == 1), stop=False,
                                    )
                                    del pts[kt - 1]
                            nc.tensor.matmul(
                                at[:],
                                vp[:, 15, h2 * VD : (h2 + 1) * VD],
                                pts[15][:],
                                start=False, stop=True,
                            )
                            nc.vector.tensor_tensor(
                                dsum[:], dsum[:], dsum2[:], ALU.add
                            )
                            pending.append((dsum, at, qh))
                            if len(pending) == 2:
                                finalize(pending.pop(0))
                        if hp + 2 < 4:
                            kvp[hp + 2] = preamble(hp + 2)
                    # ---- P4: o_proj, inside the attention pools so the
                    # last two heads' finalizes hide behind the first chain
                    for n in range(4):
                        ow = ow_tiles[n]
                        for mt in range(4):
                            last = n == 3 and mt == 3
                            if not last:
                                ps = scps.tile([P, TQ], F32, tag="sc",
                                               name="ops")
                                for h in range(NH):
                                    if h == 14 and pending:
                                        while pending:
                                            finalize(pending.pop(0))
                                    nc.tensor.matmul(
                                        ps[:],
                                        attn_sb[:, h, mt * P : (mt + 1) * P],
                                        ow[:, h, :],
                                        start=(h == 0), stop=(h == 15),
                                    )
                                st = p3q.tile([P, TQ], F32, tag="st",
                                              name="st")
                                nc.scalar.copy(st[:], ps[:])
                                nc.sync.dma_start(
                                    out_d[mt * P : (mt + 1) * P,
                                          n * 512 : (n + 1) * 512],
                                    st[:],
                                )
                                continue
                            # split the final tile so the first half's copy
                            # and store overlap the second half's matmuls
                            for c0, c1 in ((0, 384), (384, 512)):
                                w = c1 - c0
                                ps = scps.tile([P, TQ], F32, tag="sc",
                                               name="ops2")
                                for h in range(NH):
                                    nc.tensor.matmul(
                                        ps[:, :w],
                                        attn_sb[:, h, mt * P : (mt + 1) * P],
                                        ow[:, h, c0:c1],
                                        start=(h == 0), stop=(h == 15),
                                    )
                                st = p3q.tile([P, TQ], F32, tag="st",
                                              name="st2")
                                nc.scalar.copy(st[:, :w], ps[:, :w])
                                nc.sync.dma_start(
                                    out_d[mt * P : (mt + 1) * P,
                                          n * 512 + c0 : n * 512 + c1],
                                    st[:, :w],
                                )

    nc.finalize()
    return nc


def _host_prep(inputs):
    import ml_dtypes

    BF = ml_dtypes.bfloat16

    def bf(a):
        return np.ascontiguousarray(np.asarray(a, dtype=np.float32).astype(BF))

    x = np.asarray(inputs["hidden_states"], dtype=np.float32)

    qa_w = np.asarray(inputs["q_a_w"], np.float32)  # [HID, Q_RANK]
    qa_t = bf(qa_w.reshape(16, P, 12, P).transpose(1, 2, 0, 3))

    # fold the q rmsnorm weight (and the sqrt(rank) factor of the mean)
    # into the q_b rows; the per-token rsqrt is applied after q_b on-device
    lnq = (np.asarray(inputs["q_a_ln_w"], np.float64) * math.sqrt(Q_RANK)).astype(
        np.float32
    )
    qb = np.asarray(inputs["q_b_w"], np.float32) * lnq[:, None]
    qb = qb.reshape(Q_RANK, NH, HD)
    nope_cols = qb[:, :, :NOPE].reshape(Q_RANK, NH * NOPE)
    rope1 = qb[:, :, NOPE : NOPE + 32].reshape(Q_RANK, NH * 32)
    rope2 = qb[:, :, NOPE + 32 :].reshape(Q_RANK, NH * 32)
    # rope halves in rounds of 8 heads: q1(h0-7), q2(h0-7), q1(h8-15), q2(h8-15)
    qb_cols = np.concatenate(
        [nope_cols, rope1[:, :256], rope2[:, :256], rope1[:, 256:], rope2[:, 256:]],
        axis=1,
    )  # [1536, 3072]
    qb_t = bf(qb_cols.reshape(12, P, 24, P).transpose(1, 2, 0, 3))

    kva = np.asarray(inputs["kv_a_w"], np.float32)
    lat = kva[:, :KV_RANK]
    krope = kva[:, KV_RANK:].reshape(HID, NKV, ROPE)
    kr1 = krope[:, :, :32].reshape(HID, NKV * 32)
    kr2 = krope[:, :, 32:].reshape(HID, NKV * 32)
    kva_cols = np.concatenate([lat, kr1, kr2], axis=1)  # [2048, 1024]
    kva_t = bf(kva_cols.reshape(16, P, 1024).transpose(1, 0, 2))

    kvb = np.asarray(inputs["kv_b_w"], np.float32).reshape(KV_RANK, NKV, NOPE + VD)
    knope_cols = kvb[:, :, :NOPE].reshape(KV_RANK, NKV * NOPE)
    v_cols = kvb[:, :, NOPE:].reshape(KV_RANK, NKV * VD)
    kvb_cols = np.concatenate([knope_cols, v_cols], axis=1)  # [512, 2048]
    kvb_t = bf(kvb_cols.reshape(4, P, 2048).transpose(1, 0, 2))

    o_w = np.asarray(inputs["o_w"], np.float32)  # [NH*VD, HID]
    o_t = bf(o_w.reshape(16, P, 4, 512).transpose(1, 2, 0, 3))

    lnkv = (
        (np.asarray(inputs["kv_a_ln_w"], np.float64) * math.sqrt(KV_RANK))
        .astype(np.float32)
        .reshape(4, P)
        .T
    )
    tbl = np.empty((P, 134), np.float32)
    tbl[:, 0:128] = 1.0
    tbl[:, 128:132] = lnkv
    tbl[:, 132] = EPS * KV_RANK
    tbl[:, 133] = EPS * Q_RANK

    inv_freq = 1.0 / (THETA ** (np.arange(0, ROPE, 2, dtype=np.float32) / ROPE))
    t = np.arange(T, dtype=np.float32)
    freqs = np.outer(t, inv_freq).astype(np.float32)
    cosk = np.tile(np.cos(freqs).T, (4, 1))  # [128, T]
    sink = np.tile(np.sin(freqs).T, (4, 1))
    cosk_b, sink_b = bf(cosk), bf(sink)
    ones_b = np.ones((P, P), BF)
    qperm = np.zeros((4, P, P), np.float32)
    for q in list(range(0, 32)) + list(range(64, 96)):
        qperm[0, q, q] = 1.0
        qperm[1, q, q + 32] = 1.0
    for q in list(range(32, 64)) + list(range(96, 128)):
        qperm[2, q, q - 32] = 1.0
        qperm[3, q, q] = 1.0
    qperm_t = bf(qperm.transpose(1, 0, 2))

    in_maps = []
    for c in range(NCORES):
        b, qc = c // 4, c % 4
        xTb = x[b].T  # [HID, T]
        qoff = qc * TQ
        xq_t = bf(xTb[:, qoff : qoff + TQ].reshape(16, P, TQ).transpose(1, 0, 2))
        in_maps.append(
            {
                "xq": xq_t,
                "qa_w": qa_t,
                "qb_w": qb_t,
                "kva_w": kva_t,
                "kvb_w": kvb_t,
                "o_w": o_t,
                "cosq": np.ascontiguousarray(cosk_b[:, qoff : qoff + TQ]),
                "sinq": np.ascontiguousarray(sink_b[:, qoff : qoff + TQ]),
                "ones_b": ones_b,
                "tbl": tbl,
                "qperm": qperm_t,
            }
        )
    return in_maps


def get_nc():
    if "nc" not in _CACHE:
        _CACHE["nc"] = _build_nc()
    return _CACHE["nc"]


def kernel(**inputs) -> np.ndarray:
    from concourse.bass_utils import run_bass_kernel_spmd

    nc = get_nc()
    in_maps = _host_prep(inputs)
    res = run_bass_kernel_spmd(nc, in_maps, core_ids=list(range(NCORES)))
    _CACHE["last_result"] = res
    outs = [res.results[c]["out"] for c in range(NCORES)]
    full = np.stack(
        [np.concatenate([outs[b * 4 + qc] for qc in range(4)], axis=0) for b in range(B)]
    )
    return full.astype(np.float32)


# revision 49
# speedup vs baseline: 1.0068x; 1.0055x over previous
"""Multi-head latent attention (MLA) TRN2 kernel.

Sharding: batch(2) x query-sequence(4) over 8 cores. Each core:
  - runs the low-rank KV projection (kv_a + rmsnorm + rope rotation)
    for ONLY its own 512 tokens, then AllGathers the scaled latents and
    rope-paired keys across the 4 cores of its batch (replica groups
    [[0..3],[4..7]])
  - computes the Q path (q_a, rmsnorm, q_b, rope) for its 512 queries
  - kv_b + full attention for its 512 queries x 2048 keys x 16 heads
  - o_proj for its chunk -> output slice [512, 2048]
Host assembles the 8 slices into [B, T, HID].

Phase order hides all five collectives under compute:
  PE warmup (p-state ramp burn during the initial DMAs) ->
  kv_a latents (k-outer sweep streaming weight chunks) -> latent
  AllGather issued ~20us in -> kv_a rope + rotation + paired scatter ->
  FOUR slot-wise rope-key AllGathers (so the earliest kv-head pairs land
  before attention needs them) -> q_a -> q_b (rope heads first) ->
  kv_b preamble pipelined per head-pair, double-buffered, interleaved
  with attention -> attention -> o_proj.

Queue discipline matters in the cost model: the HWDGE descriptor rings
are shared, so a DMA that waits long (e.g. a gather land waiting on its
collective) poisons ring slots that later weight loads recycle through.
All collective-dependent lands therefore ride the SWDGE (gpsimd/Pool)
queue, placed in Pool program order so nothing time-critical queues
behind a long wait. Weight tiles stream on the ACT queue; xq + output
stores on the SP queue.

Matmul operands are bf16 except the decoupled-rope score slice: the
rotated rope halves of q and k are stored as fp8(e4m3) in a
[32 x 2 x tokens] layout so each rope score matmul runs as a single
fp8 DoubleRow matmul (two packed 32-row k-tiles, 0.5 cycles/row, the
full 64-dim rope contraction in half the cycles of a bf16 issue). PSUM
accumulation and the softmax statistics stay f32; only the rope slice
(1/3 of the score variance) sees fp8 rounding, measured 1.4e-2 max rel
err end to end.

The q rmsnorm scale is decoupled from the PE stream: ln*sqrt(rank) is
folded into the q_b rows on the host and the per-token rsqrt rides on
the PSUM->SBUF moves after q_b. Activations are feature-major
([feature, token]) so weight tiles act as lhsT directly; attention
computes scores transposed (s^T[k,q] = k^T q) so softmax needs no
transposes: exp on ACT, the denominator via an all-ones lhsT matmul
(two parallel bf16 accumulation chains per head — bf16 halves the DVE
cost that otherwise paces the attention inner loop), and P@V consumes
the transposed probabilities directly, pipelined one key-tile behind
the score stream. o_proj runs inside the attention pool scope (reusing
the score PSUM pool) so the last two heads' softmax finalizes hide
behind the first output tile's contraction.
"""

import math

import numpy as np

B, T, HID = 2, 2048, 2048
NH, NKV = 16, 8
NOPE, ROPE = 128, 64
HD = NOPE + ROPE  # 192
VD = 128
KV_RANK, Q_RANK = 512, 1536
EPS = 1e-6
THETA = 10000.0
NCORES = 8
TQ = B * T // NCORES  # 512 query tokens per core
P = 128
SCALE = 1.0 / math.sqrt(HD)

# Rope rows are stored "paired": each head's rotated rope halves (32+32
# rows) are stacked into one contiguous 64-row slot at base partition
# 64*(kvh%2), so the score-matmul lhsT(k)/rhs(q) base partitions match
# (PE only allows bases {0, 32, 64}).

_CACHE = {}


def _build_nc():
    import concourse.bass as bass  # noqa: F401
    import concourse.mybir as mybir
    from concourse import bacc
    from concourse.tile import TileContext

    F32 = mybir.dt.float32
    F32R = mybir.dt.float32r
    BF16 = mybir.dt.bfloat16
    F8 = mybir.dt.float8e4
    DR = mybir.MatmulPerfMode.DoubleRow
    AF = mybir.ActivationFunctionType
    ALU = mybir.AluOpType

    nc = bacc.Bacc(None, target_bir_lowering=False)

    xq_d = nc.dram_tensor("xq", [P, 16, TQ], BF16, kind="ExternalInput")
    qa_d = nc.dram_tensor("qa_w", [P, 12, 16, P], BF16, kind="ExternalInput")
    qb_d = nc.dram_tensor("qb_w", [P, 24, 12, P], BF16, kind="ExternalInput")
    kva_d = nc.dram_tensor("kva_w", [P, 16, 1024], BF16, kind="ExternalInput")
    kvb_d = nc.dram_tensor("kvb_w", [P, 4, 2048], BF16, kind="ExternalInput")
    o_d = nc.dram_tensor("o_w", [P, 4, 16, 512], BF16, kind="ExternalInput")
    cosq_d = nc.dram_tensor("cosq", [P, TQ], BF16, kind="ExternalInput")
    sinq_d = nc.dram_tensor("sinq", [P, TQ], BF16, kind="ExternalInput")
    onesb_d = nc.dram_tensor("ones_b", [P, P], BF16, kind="ExternalInput")
    # packed f32 tables: cols 0:128 all-ones (f32r lhsT for the softmax
    # denominator matmul), 128:132 kv ln weight * sqrt(rank), 132:134 eps
    tbl_d = nc.dram_tensor("tbl", [P, 134], F32R, kind="ExternalInput")
    qperm_d = nc.dram_tensor("qperm", [P, 4, P], BF16, kind="ExternalInput")
    # cross-core staging: this core's 512-key kv quarter + gathered full set
    kvl_p_d = nc.dram_tensor("kvl_p", [P, 4, TQ], BF16, kind="Internal")
    kprq_p_d = [nc.dram_tensor(f"kprq{j}_p", [64, 2, TQ], F8, kind="Internal")
                for j in range(4)]
    kvl_g_d = nc.dram_tensor("kvl_g", [4, P, 4, TQ], BF16, kind="Internal")
    kprg_d = [nc.dram_tensor(f"kprg{j}", [4, 64, 2, TQ], F8, kind="Internal")
              for j in range(4)]
    CC_GROUPS = [[0, 1, 2, 3], [4, 5, 6, 7]]
    out_d = nc.dram_tensor("out", [TQ, HID], F32, kind="ExternalOutput")

    with TileContext(nc) as tc:
        with tc.tile_pool(name="resident", bufs=1) as res:
            kv_latN = res.tile([P, 4, T], BF16, name="kv_latN")
            qnope = res.tile([P, NH, TQ], BF16, name="qnope")
            qrope = res.tile([64, 8, 2, TQ], F8, name="qrope")
            kpair = res.tile([64, 4, 2, T], F8, name="kpair")
            attn_sb = res.tile([P, NH, TQ], BF16, name="attn_sb")
            kvb_sb = res.tile([P, 4, 2048], BF16, name="kvb_sb")
            ones_sb = res.tile([P, P], BF16, name="ones_sb")
            tbl_sb = res.tile([P, 134], F32R, name="tbl_sb")

            # -- scat: rope-scatter sources, allocated at the TOP of SBUF
            # (side="right") so later phases' pools never overlap their
            # addresses and thus never wait on the background scatters.
            # -- pf1: kv_a/q inputs, freed before the attention phase.
            with (
                tc.tile_pool(name="scat", bufs=2, side="right") as scat,
                tc.tile_pool(name="pf1", bufs=1) as pf1,
            ):
                kva_sb = pf1.tile([P, 16, 1024], BF16, name="kva_sb")
                xq_sb = pf1.tile([P, 16, TQ], BF16, name="xq_sb")
                qperm_sb = pf1.tile([P, 4, P], BF16, name="qperm_sb")
                cosq_sb = scat.tile([P, TQ], BF16, name="cosq_sb", bufs=1)
                sinq_sb = scat.tile([P, TQ], BF16, name="sinq_sb", bufs=1)

                # ---- input streams ----
                # sync(SP) queue: xq chunks, then ONLY the collective-
                #   dependent gather lands + output stores (their sem waits
                #   hold the SP sequencer, which nothing else runs on)
                # scalar(ACT) queue: kva cols, qa/qb weight tiles, latent
                #   stage-out, odd kpr scatters, kvb, o_w
                for c in range(4):
                    nc.sync.dma_start(
                        xq_sb[:, 4 * c : 4 * c + 4, :],
                        xq_d[:, 4 * c : 4 * c + 4, :],
                    )
                    nc.scalar.dma_start(
                        kva_sb[:, 4 * c : 4 * c + 4, 0:512],
                        kva_d[:, 4 * c : 4 * c + 4, 0:512],
                    )
                for c in range(2):
                    nc.scalar.dma_start(
                        kva_sb[:, 8 * c : 8 * c + 8, 512:1024],
                        kva_d[:, 8 * c : 8 * c + 8, 512:1024],
                    )
                nc.scalar.dma_start(kvb_sb[:], kvb_d[:, :, :])
                wsrc0 = res.tile([P, 64], BF16, name="wsrc0")
                nc.vector.memset(wsrc0[:], 0.0)
                nc.gpsimd.dma_start(ones_sb[:], onesb_d[:, :])
                nc.gpsimd.dma_start(tbl_sb[:], tbl_d[:, :])
                nc.gpsimd.dma_start(cosq_sb[:], cosq_d[:, :])
                nc.gpsimd.dma_start(sinq_sb[:], sinq_d[:, :])
                nc.gpsimd.dma_start(qperm_sb[:], qperm_d[:, :, :])

                # ---- PE warmup: burn the p-state ramp during initial DMA.
                with (
                    tc.tile_pool(name="wu", bufs=1) as wu,
                    tc.tile_pool(name="wups", bufs=1, space="PSUM") as wups,
                ):
                    wps = wups.tile([P, 64], F32, tag="wu")
                    for _ in range(96):
                        nc.tensor.matmul(
                            wps[0:64, :], wsrc0[:, :], wsrc0[:, :],
                            start=True, stop=True,
                        )

                # ------------- P1: kv_a for THIS core's 512 tokens ---------
                # Latent sweep first (k-outer so matmuls consume weight
                # chunks as they land), stage + AllGather A. Then the rope
                # sweep, rotation, paired scatter, AllGather B.
                with (
                    tc.tile_pool(name="p1l", bufs=1) as p1l,
                    tc.tile_pool(name="p1s", bufs=2) as p1s,
                    tc.tile_pool(name="p1ps", bufs=1, space="PSUM") as p1ps,
                    tc.tile_pool(name="p1ps1", bufs=1, space="PSUM") as p1ps1,
                ):
                    kvl_loc = p1l.tile([P, 4, TQ], BF16, name="kvl_loc")
                    raw1 = p1l.tile([P, 2, TQ], BF16, name="raw1")
                    raw2 = p1l.tile([P, 2, TQ], BF16, name="raw2")

                    # latent sweep: 4 live psum accumulators (m 0..3), k-outer
                    # so matmuls consume kva weight chunks as they land
                    lps = {}
                    for m in range(4):
                        lps[m] = p1ps.tile([P, TQ], F32, tag=f"kl{m}",
                                           name=f"kl{m}")
                    for k in range(16):
                        for m in range(4):
                            nc.tensor.matmul(
                                lps[m][:],
                                kva_sb[:, k, m * P : (m + 1) * P],
                                xq_sb[:, k, :],
                                start=(k == 0), stop=(k == 15),
                            )
                    ksumsq = p1ps1.tile([P, TQ], F32, tag="ksumsq")
                    for m in range(4):
                        nc.scalar.copy(kvl_loc[:, m, :], lps[m][:])
                    for m in range(4):
                        sq = p1s.tile([P, TQ], BF16, tag="ksq")
                        nc.vector.tensor_tensor(
                            sq[:], kvl_loc[:, m, :], kvl_loc[:, m, :], ALU.mult
                        )
                        nc.tensor.matmul(
                            ksumsq[:], ones_sb[:], sq[:],
                            start=(m == 0), stop=(m == 3),
                        )
                    ksqt = p1s.tile([P, TQ], F32, tag="ksqt", bufs=1)
                    nc.scalar.activation(
                        ksqt[:], ksumsq[:], AF.Sqrt, bias=tbl_sb[:, 132:133]
                    )
                    krs = p1s.tile([P, TQ], F32, tag="krs", bufs=1)
                    nc.vector.reciprocal(krs[:], ksqt[:])
                    for m in range(4):
                        nc.vector.scalar_tensor_tensor(
                            kvl_loc[:, m, :], kvl_loc[:, m, :],
                            tbl_sb[:, 128 + m : 129 + m], krs[:],
                            ALU.mult, ALU.mult,
                        )
                    nc.scalar.dma_start(kvl_p_d[:, :, :], kvl_loc[:, :, :])
                    nc.gpsimd.collective_compute(
                        "AllGather", ALU.bypass, CC_GROUPS,
                        ins=[kvl_p_d[:, :, :]], outs=[kvl_g_d[:, :, :, :]],
                    )

                    # rope sweep (kva cols 512:1024 -> m 4..7), reuses the
                    # latent psum tags
                    rps = {}
                    for m in range(4):
                        tg = f"kl{m}" if m < 2 else f"kr{m}"
                        rps[m] = p1ps.tile([P, TQ], F32, tag=tg,
                                           name=f"kr{m}")
                    for m in range(4):
                        for k in range(16):
                            nc.tensor.matmul(
                                rps[m][:],
                                kva_sb[:, k, 512 + m * P : 512 + (m + 1) * P],
                                xq_sb[:, k, :],
                                start=(k == 0), stop=(k == 15),
                            )
                    for m in range(4):
                        dst = raw1 if m < 2 else raw2
                        nc.scalar.copy(dst[:, m % 2, :], rps[m][:])

                    # rotate own keys (the rope tables for them are the
                    # query tables) and scatter straight to the DRAM part
                    ckb = cosq_sb[:, None, :].to_broadcast((P, 2, TQ))
                    skb = sinq_sb[:, None, :].to_broadcast((P, 2, TQ))
                    rt = scat.tile([P, 2, TQ], BF16, tag="rtmp", bufs=2)
                    r1 = scat.tile([P, 2, TQ], F8, tag="krot1", bufs=1)
                    rc1 = scat.tile([P, 2, TQ], BF16, tag="rc1", bufs=1)
                    nc.vector.tensor_tensor(rt[:], raw2[:], skb, ALU.mult)
                    nc.vector.tensor_tensor(rc1[:], raw1[:], ckb, ALU.mult)
                    nc.vector.tensor_tensor(r1[:], rc1[:], rt[:], ALU.subtract)
                    rt2 = scat.tile([P, 2, TQ], BF16, tag="rtmp", bufs=2)
                    rc2 = scat.tile([P, 2, TQ], BF16, tag="rc2", bufs=1)
                    r2 = scat.tile([P, 2, TQ], F8, tag="krot2", bufs=1)
                    nc.vector.tensor_tensor(rt2[:], raw1[:], skb, ALU.mult)
                    nc.vector.tensor_tensor(rc2[:], raw2[:], ckb, ALU.mult)
                    nc.vector.tensor_tensor(r2[:], rc2[:], rt2[:], ALU.add)
                    # head kvh -> slot kvh//2, base 64*(kvh%2)
                    for kvh in range(NKV):
                        t_, i = kvh // 4, kvh % 4
                        bb = 32 * (kvh % 2)
                        eng = nc.gpsimd if kvh % 2 == 0 else nc.scalar
                        kprd = kprq_p_d[kvh // 2]
                        eng.dma_start(
                            kprd[bb : bb + 32, 0, :],
                            r1[i * 32 : (i + 1) * 32, t_, :],
                        )
                        eng.dma_start(
                            kprd[bb : bb + 32, 1, :],
                            r2[i * 32 : (i + 1) * 32, t_, :],
                        )
                    for j in range(4):
                        nc.gpsimd.collective_compute(
                            "AllGather", ALU.bypass, CC_GROUPS,
                            ins=[kprq_p_d[j][:, :, :]], outs=[kprg_d[j][:, :, :, :]],
                        )
                    # land the gathered latents on the SWDGE queue: its ring
                    # is private, so the wait on the collective cannot poison
                    # the HWDGE rings the weight streams recycle through
                    for r in range(4):
                        ksl = slice(r * TQ, (r + 1) * TQ)
                        nc.gpsimd.dma_start(
                            kv_latN[:, :, ksl], kvl_g_d[r, :, :, :]
                        )

                # ------------- P2: q path -------------
                with (
                    tc.tile_pool(name="p2", bufs=1) as p2,
                    tc.tile_pool(name="p2w", bufs=4) as p2w,
                    tc.tile_pool(name="p2s", bufs=2) as p2s,
                    tc.tile_pool(name="p2ps", bufs=2, space="PSUM") as p2ps,
                    tc.tile_pool(name="p2ps1", bufs=1, space="PSUM") as p2ps1,
                ):
                    q_lat = p2.tile([P, 12, TQ], BF16, name="q_lat")
                    rs_q = p2.tile([P, TQ], F32, name="rs_q")

                    # q_a + rmsnorm statistics (ln*rs applied after q_b:
                    # ln is folded into the q_b rows on the host, rs is a
                    # per-token scale that commutes with q_b)
                    sumsq = p2ps1.tile([P, TQ], F32, tag="qsumsq")
                    for m in range(12):
                        wt = p2w.tile([P, 16, P], BF16, tag="w", name="wt")
                        nc.scalar.dma_start(wt[:], qa_d[:, m, :, :])
                        ps = p2ps.tile([P, TQ], F32, tag="mm", name="ps")
                        for k in range(16):
                            nc.tensor.matmul(
                                ps[:], wt[:, k, :], xq_sb[:, k, :],
                                start=(k == 0), stop=(k == 15),
                            )
                        nc.vector.tensor_copy(q_lat[:, m, :], ps[:])
                        sq = p2s.tile([P, TQ], BF16, tag="sq", name="sq", bufs=1)
                        if m < 2:
                            nc.vector.tensor_tensor(
                                sq[:], q_lat[:, m, :], q_lat[:, m, :], ALU.mult
                            )
                        else:
                            nc.scalar.square(sq[:], ps[:])
                        nc.tensor.matmul(
                            sumsq[:], ones_sb[:], sq[:],
                            start=(m == 0), stop=(m == 11),
                        )
                    sqt = p2s.tile([P, TQ], F32, tag="sqt", bufs=1)
                    nc.scalar.activation(
                        sqt[:], sumsq[:], AF.Sqrt, bias=tbl_sb[:, 133:134]
                    )
                    nc.vector.reciprocal(rs_q[:], sqt[:])

                    # q_b: nope heads to qnope, rope raw kept for rotation;
                    # the rs_q normalization rides on the PSUM->SBUF move.
                    # Host orders the rope halves in rounds of 8 heads:
                    # m=16,17: q1(h0-7), m=18,19: q2(h0-7),
                    # m=20,21: q1(h8-15), m=22,23: q2(h8-15).
                    qraws = {}
                    for m in list(range(16, 24)) + list(range(16)):
                        wt = p2w.tile([P, 16, P], BF16, tag="w")
                        nc.scalar.dma_start(wt[:, :12, :], qb_d[:, m, :, :])
                        ps = p2ps.tile([P, TQ], F32, tag="mm")
                        for k in range(12):
                            nc.tensor.matmul(
                                ps[:], wt[:, k, :], q_lat[:, k, :],
                                start=(k == 0), stop=(k == 11),
                            )
                        if m < 16:
                            dst = qnope[:, m, :]
                        else:
                            j = m - 16
                            half, idx = (j // 2) % 2, j % 2
                            if idx == 0:
                                qraws[half] = scat.tile(
                                    [P, 2, TQ], BF16, tag=f"qraw{half}",
                                    bufs=2, name=f"qraw{half}",
                                )
                            dst = qraws[half][:, idx, :]
                        nc.vector.tensor_tensor(dst, ps[:], rs_q[:], ALU.mult)
                        if m >= 16 and m % 4 == 3:
                            # rotate this round's 8 heads and scatter to the
                            # paired layout via the GPSIMD (SWDGE) queue
                            rnd = (m - 16) // 4
                            cb = cosq_sb[:, None, :].to_broadcast((P, 2, TQ))
                            sb_ = sinq_sb[:, None, :].to_broadcast((P, 2, TQ))
                            qr1, qr2 = qraws[0], qraws[1]
                            qt = scat.tile([P, 2, TQ], BF16, tag="qrtmp", bufs=2)
                            qc1 = scat.tile([P, 2, TQ], BF16, tag="qc1", bufs=2)
                            qo1 = scat.tile([P, 2, TQ], F8, tag="qrot1")
                            qo2 = scat.tile([P, 2, TQ], F8, tag="qrot2")
                            nc.vector.tensor_tensor(qt[:], qr2[:], sb_, ALU.mult)
                            nc.vector.tensor_tensor(qc1[:], qr1[:], cb, ALU.mult)
                            nc.vector.tensor_tensor(qo1[:], qc1[:], qt[:], ALU.subtract)
                            qt2 = scat.tile([P, 2, TQ], BF16, tag="qrtmp", bufs=2)
                            qc2 = scat.tile([P, 2, TQ], BF16, tag="qc2", bufs=2)
                            nc.vector.tensor_tensor(qt2[:], qr1[:], sb_, ALU.mult)
                            nc.vector.tensor_tensor(qc2[:], qr2[:], cb, ALU.mult)
                            nc.vector.tensor_tensor(qo2[:], qc2[:], qt2[:], ALU.add)
                            # head qh -> slot 2*(qh//4)+qh%2, base 64*((qh//2)%2)
                            for qh in range(8 * rnd, 8 * rnd + 8):
                                slot = 2 * (qh // 4) + qh % 2
                                bb = 32 * ((qh // 2) % 2)
                                src_r = (qh % 4) * 32
                                src_t = (qh % 8) // 4
                                nc.gpsimd.dma_start(
                                    qrope[bb : bb + 32, slot, 0, :],
                                    qo1[src_r : src_r + 32, src_t, :],
                                )
                                nc.gpsimd.dma_start(
                                    qrope[bb : bb + 32, slot, 1, :],
                                    qo2[src_r : src_r + 32, src_t, :],
                                )

                    # land the gathered rope-key quarters (SWDGE queue,
                    # after the qrope scatters in Pool order; each slot only
                    # waits its own collective)
                    for j in range(4):
                        for r in range(4):
                            ksl = slice(r * TQ, (r + 1) * TQ)
                            nc.gpsimd.dma_start(
                                kpair[:, j, :, ksl], kprg_d[j][r, :, :, :]
                            )

            # ------------- P3 + P4 (pf1 SBUF freed) -----------------------
            with tc.tile_pool(name="oww", bufs=4) as oww:
                ow_tiles = {}

                def ow_load(n, eng):
                    ow = oww.tile([P, 16, 512], BF16, tag="ow")
                    eng.dma_start(ow[:], o_d[:, n, :, :])
                    ow_tiles[n] = ow

                with (
                    tc.tile_pool(name="p3", bufs=2) as p3,
                    tc.tile_pool(name="p3q", bufs=2) as p3q,
                    tc.tile_pool(name="p3p", bufs=4) as p3p,
                    tc.tile_pool(name="scps", bufs=3, space="PSUM") as scps,
                    tc.tile_pool(name="atps", bufs=2, space="PSUM") as atps,
                ):
                    pending = []

                    def finalize(item):
                        dsum, at, qh = item
                        dnp = scps.tile([P, 2, TQ], F32, tag="sc", name="dnp")
                        dn = dnp[:, 0, :]
                        nc.tensor.matmul(
                            dn, ones_sb[:], dsum[:], start=True, stop=True
                        )
                        rec = p3q.tile([P, TQ], F32, tag="rec")
                        nc.vector.reciprocal(rec[:], dn)
                        nc.vector.tensor_tensor(
                            attn_sb[:, qh, :], at[:], rec[:], ALU.mult
                        )

                    # kv_b preamble for ONE head-pair: needs only the
                    # gathered LATENTS. Pipelined 2 stages ahead of the
                    # attention loop (double-buffered), so preambles hp0+hp1
                    # bridge the window between the latent gather and the
                    # rope-key gather while hp2/hp3 hide inside attention.
                    def preamble(hp):
                        kvh0 = 2 * hp
                        knp = p3.tile([P, 2, T], BF16, tag="knp", name="knp")
                        vp = p3.tile([P, 16, 256], BF16, tag="vp", name="vp")
                        for h2 in range(2):
                            wsl = slice((kvh0 + h2) * NOPE, (kvh0 + h2 + 1) * NOPE)
                            for n4 in range(4):
                                ksl = slice(n4 * 512, (n4 + 1) * 512)
                                psp = scps.tile([P, 2, TQ], F32, tag="sc",
                                                name="psp")
                                for r in range(4):
                                    nc.tensor.matmul(
                                        psp[:, 0, :], kvb_sb[:, r, wsl],
                                        kv_latN[:, r, ksl],
                                        start=(r == 0), stop=(r == 3),
                                    )
                                nc.vector.tensor_copy(knp[:, h2, ksl],
                                                      psp[:, 0, :])
                        vsl = slice(NKV * NOPE + kvh0 * VD, NKV * NOPE + (kvh0 + 2) * VD)
                        for kt in range(16):
                            psp = scps.tile([P, 2, TQ], F32, tag="sc",
                                            name="psp2")
                            for r in range(4):
                                nc.tensor.matmul(
                                    psp[:, 0, :256],
                                    kv_latN[:, r, kt * P : (kt + 1) * P],
                                    kvb_sb[:, r, vsl],
                                    start=(r == 0), stop=(r == 3),
                                )
                            if (hp == 1 and kt >= 12) or hp >= 2:
                                nc.vector.tensor_copy(vp[:, kt, :],
                                                      psp[:, 0, :256])
                            else:
                                nc.scalar.copy(vp[:, kt, :], psp[:, 0, :256])
                        if hp > 0:
                            # o_proj weight prefetch on the ACT queue
                            ow_load(hp - 1, nc.scalar)
                        if hp == 3:
                            ow_load(3, nc.scalar)
                        return knp, vp

                    kvp = {0: preamble(0), 1: preamble(1)}
                    for hp in range(4):  # kv-head pairs
                        kvh0 = 2 * hp
                        knp, vp = kvp.pop(hp)

                        for j4 in range(4):
                            qh = 4 * hp + j4
                            kvh = qh // 2
                            h2 = kvh - kvh0
                            b = 32 * (kvh % 2)
                            slot = 2 * (qh // 4) + qh % 2
                            dsum = p3q.tile([P, TQ], BF16, tag="dsum")
                            dsum2 = p3q.tile([P, TQ], BF16, tag="dsum2")
                            at = atps.tile([P, TQ], F32, tag="at")
                            pts = {}
                            for kp in range(8):  # key-tile pairs, one exp each
                                sc = scps.tile([P, 2, TQ], F32, tag="sc")
                                for half in range(2):
                                    kt = 2 * kp + half
                                    nc.tensor.matmul(
                                        sc[:, half, :],
                                        knp[:, h2, kt * P : (kt + 1) * P],
                                        qnope[:, qh, :],
                                        start=True, stop=False,
                                    )
                                    nc.tensor.matmul(
                                        sc[:, half, :],
                                        kpair[b : b + 32, kvh // 2, :, kt * P : (kt + 1) * P],
                                        qrope[b : b + 32, slot, :, :],
                                        start=False, stop=True,
                                        perf_mode=DR,
                                    )
                                pt = p3p.tile([P, 2, TQ], BF16, tag="pt")
                                nc.scalar.activation(
                                    pt[:], sc[:], AF.Exp, scale=float(SCALE)
                                )
                                pts[kp] = pt
                                d_ = dsum if kp % 2 == 0 else dsum2
                                if kp < 2:
                                    nc.vector.tensor_copy(d_[:], pt[:, 0, :])
                                    nc.vector.tensor_tensor(
                                        d_[:], d_[:], pt[:, 1, :], ALU.add
                                    )
                                else:
                                    nc.vector.tensor_tensor(
                                        d_[:], d_[:], pt[:, 0, :], ALU.add
                                    )
                                    nc.vector.tensor_tensor(
                                        d_[:], d_[:], pt[:, 1, :], ALU.add
                                    )
                                if kp > 0:  # PV one pair behind scores
                                    for half in range(2):
                                        kt = 2 * (kp - 1) + half
                                        nc.tensor.matmul(
                                            at[:],
                                            vp[:, kt, h2 * VD : (h2 + 1) * VD],
                                            pts[kp - 1][:, half, :],
                                            start=(kt == 0), stop=False,
                                        )
                                    del pts[kp - 1]
                            for half in range(2):
                                kt = 14 + half
                                nc.tensor.matmul(
                                    at[:],
                                    vp[:, kt, h2 * VD : (h2 + 1) * VD],
                                    pts[7][:, half, :],
                                    start=False, stop=(half == 1),
                                )
                            nc.vector.tensor_tensor(
                                dsum[:], dsum[:], dsum2[:], ALU.add
                            )
                            pending.append((dsum, at, qh))
                            if len(pending) == 2:
                                finalize(pending.pop(0))
                        if hp + 2 < 4:
                            kvp[hp + 2] = preamble(hp + 2)
                    # ---- P4: o_proj, inside the attention pools so the
                    # last two heads' finalizes hide behind the first chain
                    for n in range(4):
                        ow = ow_tiles[n]
                        for mt in range(4):
                            last = n == 3 and mt == 3
                            if not last:
                                psp = scps.tile([P, 2, TQ], F32, tag="sc",
                                                name="ops")
                                ps = psp[:, 0, :]
                                for h in range(NH):
                                    if h == 14 and pending:
                                        while pending:
                                            finalize(pending.pop(0))
                                    nc.tensor.matmul(
                                        ps,
                                        attn_sb[:, h, mt * P : (mt + 1) * P],
                                        ow[:, h, :],
                                        start=(h == 0), stop=(h == 15),
                                    )
                                st = p3q.tile([P, TQ], F32, tag="st",
                                              name="st")
                                nc.scalar.copy(st[:], ps)
                                nc.sync.dma_start(
                                    out_d[mt * P : (mt + 1) * P,
                                          n * 512 : (n + 1) * 512],
                                    st[:],
                                )
                                continue
                            # split the final tile so the first half's copy
                            # and store overlap the second half's matmuls
                            for c0, c1 in ((0, 384), (384, 512)):
                                w = c1 - c0
                                psp = scps.tile([P, 2, TQ], F32, tag="sc",
                                                name="ops2")
                                for h in range(NH):
                                    nc.tensor.matmul(
                                        psp[:, 0, :w],
                                        attn_sb[:, h, mt * P : (mt + 1) * P],
                                        ow[:, h, c0:c1],
                                        start=(h == 0), stop=(h == 15),
                                    )
                                st = p3q.tile([P, TQ], F32, tag="st",
                                              name="st2")
                                nc.scalar.copy(st[:, :w], psp[:, 0, :w])
                                nc.sync.dma_start(
                                    out_d[mt * P : (mt + 1) * P,
                                          n * 512 + c0 : n * 512 + c1],
                                    st[:, :w],
                                )

    nc.finalize()
    return nc


def _host_prep(inputs):
    import ml_dtypes

    BF = ml_dtypes.bfloat16

    def bf(a):
        return np.ascontiguousarray(np.asarray(a, dtype=np.float32).astype(BF))

    x = np.asarray(inputs["hidden_states"], dtype=np.float32)

    qa_w = np.asarray(inputs["q_a_w"], np.float32)  # [HID, Q_RANK]
    qa_t = bf(qa_w.reshape(16, P, 12, P).transpose(1, 2, 0, 3))

    # fold the q rmsnorm weight (and the sqrt(rank) factor of the mean)
    # into the q_b rows; the per-token rsqrt is applied after q_b on-device
    lnq = (np.asarray(inputs["q_a_ln_w"], np.float64) * math.sqrt(Q_RANK)).astype(
        np.float32
    )
    qb = np.asarray(inputs["q_b_w"], np.float32) * lnq[:, None]
    qb = qb.reshape(Q_RANK, NH, HD)
    nope_cols = qb[:, :, :NOPE].reshape(Q_RANK, NH * NOPE)
    rope1 = qb[:, :, NOPE : NOPE + 32].reshape(Q_RANK, NH * 32)
    rope2 = qb[:, :, NOPE + 32 :].reshape(Q_RANK, NH * 32)
    # rope halves in rounds of 8 heads: q1(h0-7), q2(h0-7), q1(h8-15), q2(h8-15)
    qb_cols = np.concatenate(
        [nope_cols, rope1[:, :256], rope2[:, :256], rope1[:, 256:], rope2[:, 256:]],
        axis=1,
    )  # [1536, 3072]
    qb_t = bf(qb_cols.reshape(12, P, 24, P).transpose(1, 2, 0, 3))

    kva = np.asarray(inputs["kv_a_w"], np.float32)
    lat = kva[:, :KV_RANK]
    krope = kva[:, KV_RANK:].reshape(HID, NKV, ROPE)
    kr1 = krope[:, :, :32].reshape(HID, NKV * 32)
    kr2 = krope[:, :, 32:].reshape(HID, NKV * 32)
    kva_cols = np.concatenate([lat, kr1, kr2], axis=1)  # [2048, 1024]
    kva_t = bf(kva_cols.reshape(16, P, 1024).transpose(1, 0, 2))

    kvb = np.asarray(inputs["kv_b_w"], np.float32).reshape(KV_RANK, NKV, NOPE + VD)
    knope_cols = kvb[:, :, :NOPE].reshape(KV_RANK, NKV * NOPE)
    v_cols = kvb[:, :, NOPE:].reshape(KV_RANK, NKV * VD)
    kvb_cols = np.concatenate([knope_cols, v_cols], axis=1)  # [512, 2048]
    kvb_t = bf(kvb_cols.reshape(4, P, 2048).transpose(1, 0, 2))

    o_w = np.asarray(inputs["o_w"], np.float32)  # [NH*VD, HID]
    o_t = bf(o_w.reshape(16, P, 4, 512).transpose(1, 2, 0, 3))

    lnkv = (
        (np.asarray(inputs["kv_a_ln_w"], np.float64) * math.sqrt(KV_RANK))
        .astype(np.float32)
        .reshape(4, P)
        .T
    )
    tbl = np.empty((P, 134), np.float32)
    tbl[:, 0:128] = 1.0
    tbl[:, 128:132] = lnkv
    tbl[:, 132] = EPS * KV_RANK
    tbl[:, 133] = EPS * Q_RANK

    inv_freq = 1.0 / (THETA ** (np.arange(0, ROPE, 2, dtype=np.float32) / ROPE))
    t = np.arange(T, dtype=np.float32)
    freqs = np.outer(t, inv_freq).astype(np.float32)
    cosk = np.tile(np.cos(freqs).T, (4, 1))  # [128, T]
    sink = np.tile(np.sin(freqs).T, (4, 1))
    cosk_b, sink_b = bf(cosk), bf(sink)
    ones_b = np.ones((P, P), BF)
    qperm = np.zeros((4, P, P), np.float32)
    for q in list(range(0, 32)) + list(range(64, 96)):
        qperm[0, q, q] = 1.0
        qperm[1, q, q + 32] = 1.0
    for q in list(range(32, 64)) + list(range(96, 128)):
        qperm[2, q, q - 32] = 1.0
        qperm[3, q, q] = 1.0
    qperm_t = bf(qperm.transpose(1, 0, 2))

    in_maps = []
    for c in range(NCORES):
        b, qc = c // 4, c % 4
        xTb = x[b].T  # [HID, T]
        qoff = qc * TQ
        xq_t = bf(xTb[:, qoff : qoff + TQ].reshape(16, P, TQ).transpose(1, 0, 2))
        in_maps.append(
            {
                "xq": xq_t,
                "qa_w": qa_t,
                "qb_w": qb_t,
                "kva_w": kva_t,
                "kvb_w": kvb_t,
                "o_w": o_t,
                "cosq": np.ascontiguousarray(cosk_b[:, qoff : qoff + TQ]),
                "sinq": np.ascontiguousarray(sink_b[:, qoff : qoff + TQ]),
                "ones_b": ones_b,
                "tbl": tbl,
                "qperm": qperm_t,
            }
        )
    return in_maps


def get_nc():
    if "nc" not in _CACHE:
        _CACHE["nc"] = _build_nc()
    return _CACHE["nc"]


def kernel(**inputs) -> np.ndarray:
    from concourse.bass_utils import run_bass_kernel_spmd

    nc = get_nc()
    in_maps = _host_prep(inputs)
    res = run_bass_kernel_spmd(nc, in_maps, core_ids=list(range(NCORES)))
    _CACHE["last_result"] = res
    outs = [res.results[c]["out"] for c in range(NCORES)]
    full = np.stack(
        [np.concatenate([outs[b * 4 + qc] for qc in range(4)], axis=0) for b in range(B)]
    )
    return full.astype(np.float32)


# revision 51
# speedup vs baseline: 1.0070x; 1.0002x over previous
"""Multi-head latent attention (MLA) TRN2 kernel.

Sharding: batch(2) x query-sequence(4) over 8 cores. Each core:
  - runs the low-rank KV projection (kv_a + rmsnorm + rope rotation)
    for ONLY its own 512 tokens, then AllGathers the scaled latents and
    rope-paired keys across the 4 cores of its batch (replica groups
    [[0..3],[4..7]])
  - computes the Q path (q_a, rmsnorm, q_b, rope) for its 512 queries
  - kv_b + full attention for its 512 queries x 2048 keys x 16 heads
  - o_proj for its chunk -> output slice [512, 2048]
Host assembles the 8 slices into [B, T, HID].

Phase order hides all five collectives under compute:
  PE warmup (p-state ramp burn during the initial DMAs) ->
  kv_a latents (k-outer sweep streaming weight chunks) -> latent
  AllGather issued ~20us in -> kv_a rope + rotation + paired scatter ->
  FOUR slot-wise rope-key AllGathers (so the earliest kv-head pairs land
  before attention needs them) -> q_a -> q_b (rope heads first) ->
  kv_b preamble pipelined per head-pair, double-buffered, interleaved
  with attention -> attention -> o_proj.

Queue discipline matters in the cost model: the HWDGE descriptor rings
are shared, so a DMA that waits long (e.g. a gather land waiting on its
collective) poisons ring slots that later weight loads recycle through.
All collective-dependent lands therefore ride the SWDGE (gpsimd/Pool)
queue, placed in Pool program order so nothing time-critical queues
behind a long wait. Weight tiles stream on the ACT queue; xq + output
stores on the SP queue.

Matmul operands are bf16 except the decoupled-rope score slice: the
rotated rope halves of q and k are stored as fp8(e4m3) in a
[32 x 2 x tokens] layout so each rope score matmul runs as a single
fp8 DoubleRow matmul (two packed 32-row k-tiles, 0.5 cycles/row, the
full 64-dim rope contraction in half the cycles of a bf16 issue). PSUM
accumulation and the softmax statistics stay f32; only the rope slice
(1/3 of the score variance) sees fp8 rounding, measured 1.4e-2 max rel
err end to end.

The q rmsnorm scale is decoupled from the PE stream: ln*sqrt(rank) is
folded into the q_b rows on the host and the per-token rsqrt rides on
the PSUM->SBUF moves after q_b. Activations are feature-major
([feature, token]) so weight tiles act as lhsT directly; attention
computes scores transposed (s^T[k,q] = k^T q) so softmax needs no
transposes: exp on ACT, the denominator via an all-ones lhsT matmul
(two parallel bf16 accumulation chains per head — bf16 halves the DVE
cost that otherwise paces the attention inner loop), and P@V consumes
the transposed probabilities directly, pipelined one key-tile behind
the score stream. o_proj runs inside the attention pool scope (reusing
the score PSUM pool) so the last two heads' softmax finalizes hide
behind the first output tile's contraction.
"""

import math

import numpy as np

B, T, HID = 2, 2048, 2048
NH, NKV = 16, 8
NOPE, ROPE = 128, 64
HD = NOPE + ROPE  # 192
VD = 128
KV_RANK, Q_RANK = 512, 1536
EPS = 1e-6
THETA = 10000.0
NCORES = 8
TQ = B * T // NCORES  # 512 query tokens per core
P = 128
SCALE = 1.0 / math.sqrt(HD)

# Rope rows are stored "paired": each head's rotated rope halves (32+32
# rows) are stacked into one contiguous 64-row slot at base partition
# 64*(kvh%2), so the score-matmul lhsT(k)/rhs(q) base partitions match
# (PE only allows bases {0, 32, 64}).

_CACHE = {}


def _build_nc():
    import concourse.bass as bass  # noqa: F401
    import concourse.mybir as mybir
    from concourse import bacc
    from concourse.tile import TileContext

    F32 = mybir.dt.float32
    F32R = mybir.dt.float32r
    BF16 = mybir.dt.bfloat16
    F8 = mybir.dt.float8e4
    DR = mybir.MatmulPerfMode.DoubleRow
    AF = mybir.ActivationFunctionType
    ALU = mybir.AluOpType

    nc = bacc.Bacc(None, target_bir_lowering=False)

    xq_d = nc.dram_tensor("xq", [P, 16, TQ], BF16, kind="ExternalInput")
    qa_d = nc.dram_tensor("qa_w", [P, 12, 16, P], BF16, kind="ExternalInput")
    qb_d = nc.dram_tensor("qb_w", [P, 24, 12, P], BF16, kind="ExternalInput")
    kva_d = nc.dram_tensor("kva_w", [P, 16, 1024], BF16, kind="ExternalInput")
    kvb_d = nc.dram_tensor("kvb_w", [P, 4, 2048], BF16, kind="ExternalInput")
    o_d = nc.dram_tensor("o_w", [P, 4, 16, 512], BF16, kind="ExternalInput")
    cosq_d = nc.dram_tensor("cosq", [P, TQ], BF16, kind="ExternalInput")
    sinq_d = nc.dram_tensor("sinq", [P, TQ], BF16, kind="ExternalInput")
    onesb_d = nc.dram_tensor("ones_b", [P, P], BF16, kind="ExternalInput")
    # packed f32 tables: cols 0:128 all-ones (f32r lhsT for the softmax
    # denominator matmul), 128:132 kv ln weight * sqrt(rank), 132:134 eps
    tbl_d = nc.dram_tensor("tbl", [P, 134], F32R, kind="ExternalInput")
    qperm_d = nc.dram_tensor("qperm", [P, 4, P], BF16, kind="ExternalInput")
    # cross-core staging: this core's 512-key kv quarter + gathered full set
    kvl_p_d = nc.dram_tensor("kvl_p", [P, 4, TQ], BF16, kind="Internal")
    kprq_p_d = [nc.dram_tensor(f"kprq{j}_p", [64, 2, TQ], F8, kind="Internal")
                for j in range(4)]
    kvl_g_d = nc.dram_tensor("kvl_g", [4, P, 4, TQ], BF16, kind="Internal")
    kprg_d = [nc.dram_tensor(f"kprg{j}", [4, 64, 2, TQ], F8, kind="Internal")
              for j in range(4)]
    CC_GROUPS = [[0, 1, 2, 3], [4, 5, 6, 7]]
    out_d = nc.dram_tensor("out", [TQ, HID], F32, kind="ExternalOutput")

    with TileContext(nc) as tc:
        with tc.tile_pool(name="resident", bufs=1) as res:
            kv_latN = res.tile([P, 4, T], BF16, name="kv_latN")
            qnope = res.tile([P, NH, TQ], BF16, name="qnope")
            qrope = res.tile([64, 8, 2, TQ], F8, name="qrope")
            kpair = res.tile([64, 4, 2, T], F8, name="kpair")
            attn_sb = res.tile([P, NH, TQ], BF16, name="attn_sb")
            kvb_sb = res.tile([P, 4, 2048], BF16, name="kvb_sb")
            ones_sb = res.tile([P, P], BF16, name="ones_sb")
            tbl_sb = res.tile([P, 134], F32R, name="tbl_sb")

            # -- scat: rope-scatter sources, allocated at the TOP of SBUF
            # (side="right") so later phases' pools never overlap their
            # addresses and thus never wait on the background scatters.
            # -- pf1: kv_a/q inputs, freed before the attention phase.
            with (
                tc.tile_pool(name="scat", bufs=2, side="right") as scat,
                tc.tile_pool(name="pf1", bufs=1) as pf1,
            ):
                kva_sb = pf1.tile([P, 16, 1024], BF16, name="kva_sb")
                xq_sb = pf1.tile([P, 16, TQ], BF16, name="xq_sb")
                qperm_sb = pf1.tile([P, 4, P], BF16, name="qperm_sb")
                cosq_sb = scat.tile([P, TQ], BF16, name="cosq_sb", bufs=1)
                sinq_sb = scat.tile([P, TQ], BF16, name="sinq_sb", bufs=1)

                # ---- input streams ----
                # sync(SP) queue: xq chunks, then ONLY the collective-
                #   dependent gather lands + output stores (their sem waits
                #   hold the SP sequencer, which nothing else runs on)
                # scalar(ACT) queue: kva cols, qa/qb weight tiles, latent
                #   stage-out, odd kpr scatters, kvb, o_w
                for k0, k1 in ((0, 2), (2, 4), (4, 8), (8, 12), (12, 16)):
                    nc.sync.dma_start(
                        xq_sb[:, k0:k1, :],
                        xq_d[:, k0:k1, :],
                    )
                    nc.scalar.dma_start(
                        kva_sb[:, k0:k1, 0:512],
                        kva_d[:, k0:k1, 0:512],
                    )
                for c in range(2):
                    nc.scalar.dma_start(
                        kva_sb[:, 8 * c : 8 * c + 8, 512:1024],
                        kva_d[:, 8 * c : 8 * c + 8, 512:1024],
                    )
                nc.scalar.dma_start(kvb_sb[:], kvb_d[:, :, :])
                wsrc0 = res.tile([P, 64], BF16, name="wsrc0")
                nc.vector.memset(wsrc0[:], 0.0)
                nc.gpsimd.dma_start(ones_sb[:], onesb_d[:, :])
                nc.gpsimd.dma_start(tbl_sb[:], tbl_d[:, :])
                nc.gpsimd.dma_start(cosq_sb[:], cosq_d[:, :])
                nc.gpsimd.dma_start(sinq_sb[:], sinq_d[:, :])
                nc.gpsimd.dma_start(qperm_sb[:], qperm_d[:, :, :])

                # ---- PE warmup: burn the p-state ramp during initial DMA.
                with (
                    tc.tile_pool(name="wu", bufs=1) as wu,
                    tc.tile_pool(name="wups", bufs=1, space="PSUM") as wups,
                ):
                    wps = wups.tile([P, 64], F32, tag="wu")
                    for _ in range(72):
                        nc.tensor.matmul(
                            wps[0:64, :], wsrc0[:, :], wsrc0[:, :],
                            start=True, stop=True,
                        )

                # ------------- P1: kv_a for THIS core's 512 tokens ---------
                # Latent sweep first (k-outer so matmuls consume weight
                # chunks as they land), stage + AllGather A. Then the rope
                # sweep, rotation, paired scatter, AllGather B.
                with (
                    tc.tile_pool(name="p1l", bufs=1) as p1l,
                    tc.tile_pool(name="p1s", bufs=2) as p1s,
                    tc.tile_pool(name="p1ps", bufs=1, space="PSUM") as p1ps,
                    tc.tile_pool(name="p1ps1", bufs=1, space="PSUM") as p1ps1,
                ):
                    kvl_loc = p1l.tile([P, 4, TQ], BF16, name="kvl_loc")
                    raw1 = p1l.tile([P, 2, TQ], BF16, name="raw1")
                    raw2 = p1l.tile([P, 2, TQ], BF16, name="raw2")

                    # latent sweep: 4 live psum accumulators (m 0..3), k-outer
                    # so matmuls consume kva weight chunks as they land
                    lps = {}
                    for m in range(4):
                        lps[m] = p1ps.tile([P, TQ], F32, tag=f"kl{m}",
                                           name=f"kl{m}")
                    for k in range(16):
                        for m in range(4):
                            nc.tensor.matmul(
                                lps[m][:],
                                kva_sb[:, k, m * P : (m + 1) * P],
                                xq_sb[:, k, :],
                                start=(k == 0), stop=(k == 15),
                            )
                    ksumsq = p1ps1.tile([P, TQ], F32, tag="ksumsq")
                    for m in range(4):
                        nc.scalar.copy(kvl_loc[:, m, :], lps[m][:])
                    for m in range(4):
                        sq = p1s.tile([P, TQ], BF16, tag="ksq")
                        nc.vector.tensor_tensor(
                            sq[:], kvl_loc[:, m, :], kvl_loc[:, m, :], ALU.mult
                        )
                        nc.tensor.matmul(
                            ksumsq[:], ones_sb[:], sq[:],
                            start=(m == 0), stop=(m == 3),
                        )
                    ksqt = p1s.tile([P, TQ], F32, tag="ksqt", bufs=1)
                    nc.scalar.activation(
                        ksqt[:], ksumsq[:], AF.Sqrt, bias=tbl_sb[:, 132:133]
                    )
                    krs = p1s.tile([P, TQ], F32, tag="krs", bufs=1)
                    nc.vector.reciprocal(krs[:], ksqt[:])
                    for m in range(4):
                        nc.vector.scalar_tensor_tensor(
                            kvl_loc[:, m, :], kvl_loc[:, m, :],
                            tbl_sb[:, 128 + m : 129 + m], krs[:],
                            ALU.mult, ALU.mult,
                        )
                    nc.scalar.dma_start(kvl_p_d[:, :, :], kvl_loc[:, :, :])
                    nc.gpsimd.collective_compute(
                        "AllGather", ALU.bypass, CC_GROUPS,
                        ins=[kvl_p_d[:, :, :]], outs=[kvl_g_d[:, :, :, :]],
                    )

                    # rope sweep (kva cols 512:1024 -> m 4..7), reuses the
                    # latent psum tags
                    rps = {}
                    for m in range(4):
                        tg = f"kl{m}" if m < 2 else f"kr{m}"
                        rps[m] = p1ps.tile([P, TQ], F32, tag=tg,
                                           name=f"kr{m}")
                    for m in range(4):
                        for k in range(16):
                            nc.tensor.matmul(
                                rps[m][:],
                                kva_sb[:, k, 512 + m * P : 512 + (m + 1) * P],
                                xq_sb[:, k, :],
                                start=(k == 0), stop=(k == 15),
                            )
                    for m in range(4):
                        dst = raw1 if m < 2 else raw2
                        nc.scalar.copy(dst[:, m % 2, :], rps[m][:])

                    # rotate own keys (the rope tables for them are the
                    # query tables) and scatter straight to the DRAM part
                    ckb = cosq_sb[:, None, :].to_broadcast((P, 2, TQ))
                    skb = sinq_sb[:, None, :].to_broadcast((P, 2, TQ))
                    rt = scat.tile([P, 2, TQ], BF16, tag="rtmp", bufs=2)
                    r1 = scat.tile([P, 2, TQ], F8, tag="krot1", bufs=1)
                    rc1 = scat.tile([P, 2, TQ], BF16, tag="rc1", bufs=1)
                    nc.vector.tensor_tensor(rt[:], raw2[:], skb, ALU.mult)
                    nc.vector.tensor_tensor(rc1[:], raw1[:], ckb, ALU.mult)
                    nc.vector.tensor_tensor(r1[:], rc1[:], rt[:], ALU.subtract)
                    rt2 = scat.tile([P, 2, TQ], BF16, tag="rtmp", bufs=2)
                    rc2 = scat.tile([P, 2, TQ], BF16, tag="rc2", bufs=1)
                    r2 = scat.tile([P, 2, TQ], F8, tag="krot2", bufs=1)
                    nc.vector.tensor_tensor(rt2[:], raw1[:], skb, ALU.mult)
                    nc.vector.tensor_tensor(rc2[:], raw2[:], ckb, ALU.mult)
                    nc.vector.tensor_tensor(r2[:], rc2[:], rt2[:], ALU.add)
                    # head kvh -> slot kvh//2, base 64*(kvh%2)
                    for kvh in range(NKV):
                        t_, i = kvh // 4, kvh % 4
                        bb = 32 * (kvh % 2)
                        eng = nc.gpsimd if kvh % 2 == 0 else nc.scalar
                        kprd = kprq_p_d[kvh // 2]
                        eng.dma_start(
                            kprd[bb : bb + 32, 0, :],
                            r1[i * 32 : (i + 1) * 32, t_, :],
                        )
                        eng.dma_start(
                            kprd[bb : bb + 32, 1, :],
                            r2[i * 32 : (i + 1) * 32, t_, :],
                        )
                    for j in range(4):
                        nc.gpsimd.collective_compute(
                            "AllGather", ALU.bypass, CC_GROUPS,
                            ins=[kprq_p_d[j][:, :, :]], outs=[kprg_d[j][:, :, :, :]],
                        )
                    # land the gathered latents on the SWDGE queue: its ring
                    # is private, so the wait on the collective cannot poison
                    # the HWDGE rings the weight streams recycle through
                    for r in range(4):
                        ksl = slice(r * TQ, (r + 1) * TQ)
                        nc.gpsimd.dma_start(
                            kv_latN[:, :, ksl], kvl_g_d[r, :, :, :]
                        )

                # ------------- P2: q path -------------
                with (
                    tc.tile_pool(name="p2", bufs=1) as p2,
                    tc.tile_pool(name="p2w", bufs=4) as p2w,
                    tc.tile_pool(name="p2s", bufs=2) as p2s,
                    tc.tile_pool(name="p2ps", bufs=2, space="PSUM") as p2ps,
                    tc.tile_pool(name="p2ps1", bufs=1, space="PSUM") as p2ps1,
                ):
                    q_lat = p2.tile([P, 12, TQ], BF16, name="q_lat")
                    rs_q = p2.tile([P, TQ], F32, name="rs_q")

                    # q_a + rmsnorm statistics (ln*rs applied after q_b:
                    # ln is folded into the q_b rows on the host, rs is a
                    # per-token scale that commutes with q_b)
                    sumsq = p2ps1.tile([P, TQ], F32, tag="qsumsq")
                    for m in range(12):
                        wt = p2w.tile([P, 16, P], BF16, tag="w", name="wt")
                        nc.scalar.dma_start(wt[:], qa_d[:, m, :, :])
                        ps = p2ps.tile([P, TQ], F32, tag="mm", name="ps")
                        for k in range(16):
                            nc.tensor.matmul(
                                ps[:], wt[:, k, :], xq_sb[:, k, :],
                                start=(k == 0), stop=(k == 15),
                            )
                        nc.vector.tensor_copy(q_lat[:, m, :], ps[:])
                        sq = p2s.tile([P, TQ], BF16, tag="sq", name="sq", bufs=1)
                        if m < 2:
                            nc.vector.tensor_tensor(
                                sq[:], q_lat[:, m, :], q_lat[:, m, :], ALU.mult
                            )
                        else:
                            nc.scalar.square(sq[:], ps[:])
                        nc.tensor.matmul(
                            sumsq[:], ones_sb[:], sq[:],
                            start=(m == 0), stop=(m == 11),
                        )
                    sqt = p2s.tile([P, TQ], F32, tag="sqt", bufs=1)
                    nc.scalar.activation(
                        sqt[:], sumsq[:], AF.Sqrt, bias=tbl_sb[:, 133:134]
                    )
                    nc.vector.reciprocal(rs_q[:], sqt[:])

                    # q_b: nope heads to qnope, rope raw kept for rotation;
                    # the rs_q normalization rides on the PSUM->SBUF move.
                    # Host orders the rope halves in rounds of 8 heads:
                    # m=16,17: q1(h0-7), m=18,19: q2(h0-7),
                    # m=20,21: q1(h8-15), m=22,23: q2(h8-15).
                    qraws = {}
                    for m in list(range(16, 24)) + list(range(16)):
                        wt = p2w.tile([P, 16, P], BF16, tag="w")
                        nc.scalar.dma_start(wt[:, :12, :], qb_d[:, m, :, :])
                        ps = p2ps.tile([P, TQ], F32, tag="mm")
                        for k in range(12):
                            nc.tensor.matmul(
                                ps[:], wt[:, k, :], q_lat[:, k, :],
                                start=(k == 0), stop=(k == 11),
                            )
                        if m < 16:
                            dst = qnope[:, m, :]
                        else:
                            j = m - 16
                            half, idx = (j // 2) % 2, j % 2
                            if idx == 0:
                                qraws[half] = scat.tile(
                                    [P, 2, TQ], BF16, tag=f"qraw{half}",
                                    bufs=2, name=f"qraw{half}",
                                )
                            dst = qraws[half][:, idx, :]
                        nc.vector.tensor_tensor(dst, ps[:], rs_q[:], ALU.mult)
                        if m >= 16 and m % 4 == 3:
                            # rotate this round's 8 heads and scatter to the
                            # paired layout via the GPSIMD (SWDGE) queue
                            rnd = (m - 16) // 4
                            cb = cosq_sb[:, None, :].to_broadcast((P, 2, TQ))
                            sb_ = sinq_sb[:, None, :].to_broadcast((P, 2, TQ))
                            qr1, qr2 = qraws[0], qraws[1]
                            qt = scat.tile([P, 2, TQ], BF16, tag="qrtmp", bufs=2)
                            qc1 = scat.tile([P, 2, TQ], BF16, tag="qc1", bufs=2)
                            qo1 = scat.tile([P, 2, TQ], F8, tag="qrot1")
                            qo2 = scat.tile([P, 2, TQ], F8, tag="qrot2")
                            nc.vector.tensor_tensor(qt[:], qr2[:], sb_, ALU.mult)
                            nc.vector.tensor_tensor(qc1[:], qr1[:], cb, ALU.mult)
                            nc.vector.tensor_tensor(qo1[:], qc1[:], qt[:], ALU.subtract)
                            qt2 = scat.tile([P, 2, TQ], BF16, tag="qrtmp", bufs=2)
                            qc2 = scat.tile([P, 2, TQ], BF16, tag="qc2", bufs=2)
                            nc.vector.tensor_tensor(qt2[:], qr1[:], sb_, ALU.mult)
                            nc.vector.tensor_tensor(qc2[:], qr2[:], cb, ALU.mult)
                            nc.vector.tensor_tensor(qo2[:], qc2[:], qt2[:], ALU.add)
                            # head qh -> slot 2*(qh//4)+qh%2, base 64*((qh//2)%2)
                            for qh in range(8 * rnd, 8 * rnd + 8):
                                slot = 2 * (qh // 4) + qh % 2
                                bb = 32 * ((qh // 2) % 2)
                                src_r = (qh % 4) * 32
                                src_t = (qh % 8) // 4
                                nc.gpsimd.dma_start(
                                    qrope[bb : bb + 32, slot, 0, :],
                                    qo1[src_r : src_r + 32, src_t, :],
                                )
                                nc.gpsimd.dma_start(
                                    qrope[bb : bb + 32, slot, 1, :],
                                    qo2[src_r : src_r + 32, src_t, :],
                                )

                    # land the gathered rope-key quarters (SWDGE queue,
                    # after the qrope scatters in Pool order; each slot only
                    # waits its own collective)
                    for j in range(4):
                        for r in range(4):
                            ksl = slice(r * TQ, (r + 1) * TQ)
                            nc.gpsimd.dma_start(
                                kpair[:, j, :, ksl], kprg_d[j][r, :, :, :]
                            )

            # ------------- P3 + P4 (pf1 SBUF freed) -----------------------
            with tc.tile_pool(name="oww", bufs=4) as oww:
                ow_tiles = {}

                def ow_load(n, eng):
                    ow = oww.tile([P, 16, 512], BF16, tag="ow")
                    eng.dma_start(ow[:], o_d[:, n, :, :])
                    ow_tiles[n] = ow

                with (
                    tc.tile_pool(name="p3", bufs=2) as p3,
                    tc.tile_pool(name="p3q", bufs=2) as p3q,
                    tc.tile_pool(name="p3p", bufs=4) as p3p,
                    tc.tile_pool(name="scps", bufs=3, space="PSUM") as scps,
                    tc.tile_pool(name="atps", bufs=2, space="PSUM") as atps,
                ):
                    pending = []

                    def finalize(item):
                        dsum, at, qh = item
                        dnp = scps.tile([P, 2, TQ], F32, tag="sc", name="dnp")
                        dn = dnp[:, 0, :]
                        nc.tensor.matmul(
                            dn, ones_sb[:], dsum[:], start=True, stop=True
                        )
                        rec = p3q.tile([P, TQ], F32, tag="rec")
                        nc.vector.reciprocal(rec[:], dn)
                        nc.vector.tensor_tensor(
                            attn_sb[:, qh, :], at[:], rec[:], ALU.mult
                        )

                    # kv_b preamble for ONE head-pair: needs only the
                    # gathered LATENTS. Pipelined 2 stages ahead of the
                    # attention loop (double-buffered), so preambles hp0+hp1
                    # bridge the window between the latent gather and the
                    # rope-key gather while hp2/hp3 hide inside attention.
                    def preamble(hp):
                        kvh0 = 2 * hp
                        knp = p3.tile([P, 2, T], BF16, tag="knp", name="knp")
                        vp = p3.tile([P, 16, 256], BF16, tag="vp", name="vp")
                        for h2 in range(2):
                            wsl = slice((kvh0 + h2) * NOPE, (kvh0 + h2 + 1) * NOPE)
                            for n4 in range(4):
                                ksl = slice(n4 * 512, (n4 + 1) * 512)
                                psp = scps.tile([P, 2, TQ], F32, tag="sc",
                                                name="psp")
                                for r in range(4):
                                    nc.tensor.matmul(
                                        psp[:, 0, :], kvb_sb[:, r, wsl],
                                        kv_latN[:, r, ksl],
                                        start=(r == 0), stop=(r == 3),
                                    )
                                nc.vector.tensor_copy(knp[:, h2, ksl],
                                                      psp[:, 0, :])
                        vsl = slice(NKV * NOPE + kvh0 * VD, NKV * NOPE + (kvh0 + 2) * VD)
                        for kt in range(16):
                            psp = scps.tile([P, 2, TQ], F32, tag="sc",
                                            name="psp2")
                            for r in range(4):
                                nc.tensor.matmul(
                                    psp[:, 0, :256],
                                    kv_latN[:, r, kt * P : (kt + 1) * P],
                                    kvb_sb[:, r, vsl],
                                    start=(r == 0), stop=(r == 3),
                                )
                            if (hp == 1 and kt >= 12) or hp >= 2:
                                nc.vector.tensor_copy(vp[:, kt, :],
                                                      psp[:, 0, :256])
                            else:
                                nc.scalar.copy(vp[:, kt, :], psp[:, 0, :256])
                        if hp > 0:
                            # o_proj weight prefetch on the ACT queue
                            ow_load(hp - 1, nc.scalar)
                        if hp == 3:
                            ow_load(3, nc.scalar)
                        return knp, vp

                    kvp = {0: preamble(0), 1: preamble(1)}
                    for hp in range(4):  # kv-head pairs
                        kvh0 = 2 * hp
                        knp, vp = kvp.pop(hp)

                        for j4 in range(4):
                            qh = 4 * hp + j4
                            kvh = qh // 2
                            h2 = kvh - kvh0
                            b = 32 * (kvh % 2)
                            slot = 2 * (qh // 4) + qh % 2
                            dsum = p3q.tile([P, TQ], BF16, tag="dsum")
                            dsum2 = p3q.tile([P, TQ], BF16, tag="dsum2")
                            at = atps.tile([P, TQ], F32, tag="at")
                            pts = {}
                            for kp in range(8):  # key-tile pairs, one exp each
                                sc = scps.tile([P, 2, TQ], F32, tag="sc")
                                for half in range(2):
                                    kt = 2 * kp + half
                                    nc.tensor.matmul(
                                        sc[:, half, :],
                                        knp[:, h2, kt * P : (kt + 1) * P],
                                        qnope[:, qh, :],
                                        start=True, stop=False,
                                    )
                                    nc.tensor.matmul(
                                        sc[:, half, :],
                                        kpair[b : b + 32, kvh // 2, :, kt * P : (kt + 1) * P],
                                        qrope[b : b + 32, slot, :, :],
                                        start=False, stop=True,
                                        perf_mode=DR,
                                    )
                                pt = p3p.tile([P, 2, TQ], BF16, tag="pt")
                                nc.scalar.activation(
                                    pt[:], sc[:], AF.Exp, scale=float(SCALE)
                                )
                                pts[kp] = pt
                                d_ = dsum if kp % 2 == 0 else dsum2
                                if kp < 2:
                                    nc.vector.tensor_copy(d_[:], pt[:, 0, :])
                                    nc.vector.tensor_tensor(
                                        d_[:], d_[:], pt[:, 1, :], ALU.add
                                    )
                                else:
                                    nc.vector.tensor_tensor(
                                        d_[:], d_[:], pt[:, 0, :], ALU.add
                                    )
                                    nc.vector.tensor_tensor(
                                        d_[:], d_[:], pt[:, 1, :], ALU.add
                                    )
                                if kp > 0:  # PV one pair behind scores
                                    for half in range(2):
                                        kt = 2 * (kp - 1) + half
                                        nc.tensor.matmul(
                                            at[:],
                                            vp[:, kt, h2 * VD : (h2 + 1) * VD],
                                            pts[kp - 1][:, half, :],
                                            start=(kt == 0), stop=False,
                                        )
                                    del pts[kp - 1]
                            for half in range(2):
                                kt = 14 + half
                                nc.tensor.matmul(
                                    at[:],
                                    vp[:, kt, h2 * VD : (h2 + 1) * VD],
                                    pts[7][:, half, :],
                                    start=False, stop=(half == 1),
                                )
                            nc.vector.tensor_tensor(
                                dsum[:], dsum[:], dsum2[:], ALU.add
                            )
                            pending.append((dsum, at, qh))
                            if len(pending) == 2:
                                finalize(pending.pop(0))
                        if hp + 2 < 4:
                            kvp[hp + 2] = preamble(hp + 2)
                    # ---- P4: o_proj, inside the attention pools so the
                    # last two heads' finalizes hide behind the first chain
                    for n in range(4):
                        ow = ow_tiles[n]
                        for mt in range(4):
                            last = n == 3 and mt == 3
                            if not last:
                                psp = scps.tile([P, 2, TQ], F32, tag="sc",
                                                name="ops")
                                ps = psp[:, 0, :]
                                for h in range(NH):
                                    if h == 14 and pending:
                                        while pending:
                                            finalize(pending.pop(0))
                                    nc.tensor.matmul(
                                        ps,
                                        attn_sb[:, h, mt * P : (mt + 1) * P],
                                        ow[:, h, :],
                                        start=(h == 0), stop=(h == 15),
                                    )
                                st = p3q.tile([P, TQ], F32, tag="st",
                                              name="st")
                                nc.scalar.copy(st[:], ps)
                                nc.sync.dma_start(
                                    out_d[mt * P : (mt + 1) * P,
                                          n * 512 : (n + 1) * 512],
                                    st[:],
                                )
                                continue
                            # split the final tile so the first half's copy
                            # and store overlap the second half's matmuls
                            for c0, c1 in ((0, 384), (384, 512)):
                                w = c1 - c0
                                psp = scps.tile([P, 2, TQ], F32, tag="sc",
                                                name="ops2")
                                for h in range(NH):
                                    nc.tensor.matmul(
                                        psp[:, 0, :w],
                                        attn_sb[:, h, mt * P : (mt + 1) * P],
                                        ow[:, h, c0:c1],
                                        start=(h == 0), stop=(h == 15),
                                    )
                                st = p3q.tile([P, TQ], F32, tag="st",
                                              name="st2")
                                nc.scalar.copy(st[:, :w], psp[:, 0, :w])
                                nc.sync.dma_start(
                                    out_d[mt * P : (mt + 1) * P,
                                          n * 512 + c0 : n * 512 + c1],
                                    st[:, :w],
                                )

    nc.finalize()
    return nc


def _host_prep(inputs):
    import ml_dtypes

    BF = ml_dtypes.bfloat16

    def bf(a):
        return np.ascontiguousarray(np.asarray(a, dtype=np.float32).astype(BF))

    x = np.asarray(inputs["hidden_states"], dtype=np.float32)

    qa_w = np.asarray(inputs["q_a_w"], np.float32)  # [HID, Q_RANK]
    qa_t = bf(qa_w.reshape(16, P, 12, P).transpose(1, 2, 0, 3))

    # fold the q rmsnorm weight (and the sqrt(rank) factor of the mean)
    # into the q_b rows; the per-token rsqrt is applied after q_b on-device
    lnq = (np.asarray(inputs["q_a_ln_w"], np.float64) * math.sqrt(Q_RANK)).astype(
        np.float32
    )
    qb = np.asarray(inputs["q_b_w"], np.float32) * lnq[:, None]
    qb = qb.reshape(Q_RANK, NH, HD)
    nope_cols = qb[:, :, :NOPE].reshape(Q_RANK, NH * NOPE)
    rope1 = qb[:, :, NOPE : NOPE + 32].reshape(Q_RANK, NH * 32)
    rope2 = qb[:, :, NOPE + 32 :].reshape(Q_RANK, NH * 32)
    # rope halves in rounds of 8 heads: q1(h0-7), q2(h0-7), q1(h8-15), q2(h8-15)
    qb_cols = np.concatenate(
        [nope_cols, rope1[:, :256], rope2[:, :256], rope1[:, 256:], rope2[:, 256:]],
        axis=1,
    )  # [1536, 3072]
    qb_t = bf(qb_cols.reshape(12, P, 24, P).transpose(1, 2, 0, 3))

    kva = np.asarray(inputs["kv_a_w"], np.float32)
    lat = kva[:, :KV_RANK]
    krope = kva[:, KV_RANK:].reshape(HID, NKV, ROPE)
    kr1 = krope[:, :, :32].reshape(HID, NKV * 32)
    kr2 = krope[:, :, 32:].reshape(HID, NKV * 32)
    kva_cols = np.concatenate([lat, kr1, kr2], axis=1)  # [2048, 1024]
    kva_t = bf(kva_cols.reshape(16, P, 1024).transpose(1, 0, 2))

    kvb = np.asarray(inputs["kv_b_w"], np.float32).reshape(KV_RANK, NKV, NOPE + VD)
    knope_cols = kvb[:, :, :NOPE].reshape(KV_RANK, NKV * NOPE)
    v_cols = kvb[:, :, NOPE:].reshape(KV_RANK, NKV * VD)
    kvb_cols = np.concatenate([knope_cols, v_cols], axis=1)  # [512, 2048]
    kvb_t = bf(kvb_cols.reshape(4, P, 2048).transpose(1, 0, 2))

    o_w = np.asarray(inputs["o_w"], np.float32)  # [NH*VD, HID]
    o_t = bf(o_w.reshape(16, P, 4, 512).transpose(1, 2, 0, 3))

    lnkv = (
        (np.asarray(inputs["kv_a_ln_w"], np.float64) * math.sqrt(KV_RANK))
        .astype(np.float32)
        .reshape(4, P)
        .T
    )
    tbl = np.empty((P, 134), np.float32)
    tbl[:, 0:128] = 1.0
    tbl[:, 128:132] = lnkv
    tbl[:, 132] = EPS * KV_RANK
    tbl[:, 133] = EPS * Q_RANK

    inv_freq = 1.0 / (THETA ** (np.arange(0, ROPE, 2, dtype=np.float32) / ROPE))
    t = np.arange(T, dtype=np.float32)
    freqs = np.outer(t, inv_freq).astype(np.float32)
    cosk = np.tile(np.cos(freqs).T, (4, 1))  # [128, T]
    sink = np.tile(np.sin(freqs).T, (4, 1))
    cosk_b, sink_b = bf(cosk), bf(sink)
    ones_b = np.ones((P, P), BF)
    qperm = np.zeros((4, P, P), np.float32)
    for q in list(range(0, 32)) + list(range(64, 96)):
        qperm[0, q, q] = 1.0
        qperm[1, q, q + 32] = 1.0
    for q in list(range(32, 64)) + list(range(96, 128)):
        qperm[2, q, q - 32] = 1.0
        qperm[3, q, q] = 1.0
    qperm_t = bf(qperm.transpose(1, 0, 2))

    in_maps = []
    for c in range(NCORES):
        b, qc = c // 4, c % 4
        xTb = x[b].T  # [HID, T]
        qoff = qc * TQ
        xq_t = bf(xTb[:, qoff : qoff + TQ].reshape(16, P, TQ).transpose(1, 0, 2))
        in_maps.append(
            {
                "xq": xq_t,
                "qa_w": qa_t,
                "qb_w": qb_t,
                "kva_w": kva_t,
                "kvb_w": kvb_t,
                "o_w": o_t,
                "cosq": np.ascontiguousarray(cosk_b[:, qoff : qoff + TQ]),
                "sinq": np.ascontiguousarray(sink_b[:, qoff : qoff + TQ]),
                "ones_b": ones_b,
                "tbl": tbl,
                "qperm": qperm_t,
            }
        )
    return in_maps


def get_nc():
    if "nc" not in _CACHE:
        _CACHE["nc"] = _build_nc()
    return _CACHE["nc"]


def kernel(**inputs) -> np.ndarray:
    from concourse.bass_utils import run_bass_kernel_spmd

    nc = get_nc()
    in_maps = _host_prep(inputs)
    res = run_bass_kernel_spmd(nc, in_maps, core_ids=list(range(NCORES)))
    _CACHE["last_result"] = res
    outs = [res.results[c]["out"] for c in range(NCORES)]
    full = np.stack(
        [np.concatenate([outs[b * 4 + qc] for qc in range(4)], axis=0) for b in range(B)]
    )
    return full.astype(np.float32)


# revision 54
# speedup vs baseline: 1.0219x; 1.0148x over previous
"""Multi-head latent attention (MLA) TRN2 kernel.

Sharding: batch(2) x query-sequence(4) over 8 cores. Each core:
  - runs the low-rank KV projection (kv_a + rmsnorm + rope rotation)
    for ONLY its own 512 tokens, then AllGathers the scaled latents and
    rope-paired keys across the 4 cores of its batch (replica groups
    [[0..3],[4..7]])
  - computes the Q path (q_a, rmsnorm, q_b, rope) for its 512 queries
  - kv_b + full attention for its 512 queries x 2048 keys x 16 heads
  - o_proj for its chunk -> output slice [512, 2048]
Host assembles the 8 slices into [B, T, HID].

Phase order hides all five collectives under compute:
  PE warmup (p-state ramp burn during the initial DMAs) ->
  kv_a latents (k-outer sweep streaming weight chunks) -> latent
  AllGather issued ~20us in -> kv_a rope + rotation + paired scatter ->
  FOUR slot-wise rope-key AllGathers (so the earliest kv-head pairs land
  before attention needs them) -> q_a -> q_b (rope heads first) ->
  kv_b preamble pipelined per head-pair, double-buffered, interleaved
  with attention -> attention -> o_proj.

Queue discipline matters in the cost model: the HWDGE descriptor rings
are shared, so a DMA that waits long (e.g. a gather land waiting on its
collective) poisons ring slots that later weight loads recycle through.
All collective-dependent lands therefore ride the SWDGE (gpsimd/Pool)
queue, placed in Pool program order so nothing time-critical queues
behind a long wait. Weight tiles stream on the ACT queue; xq + output
stores on the SP queue.

Matmul operands are bf16 except the decoupled-rope score slice: the
rotated rope halves of q and k are stored as fp8(e4m3) in a
[32 x 2 x tokens] layout so each rope score matmul runs as a single
fp8 DoubleRow matmul (two packed 32-row k-tiles, 0.5 cycles/row, the
full 64-dim rope contraction in half the cycles of a bf16 issue). PSUM
accumulation and the softmax statistics stay f32; only the rope slice
(1/3 of the score variance) sees fp8 rounding, measured 1.4e-2 max rel
err end to end.

The q rmsnorm scale is decoupled from the PE stream: ln*sqrt(rank) is
folded into the q_b rows on the host and the per-token rsqrt rides on
the PSUM->SBUF moves after q_b. Activations are feature-major
([feature, token]) so weight tiles act as lhsT directly; attention
computes scores transposed (s^T[k,q] = k^T q) so softmax needs no
transposes: exp on ACT, the denominator via an all-ones lhsT matmul
(two parallel bf16 accumulation chains per head — bf16 halves the DVE
cost that otherwise paces the attention inner loop), and P@V consumes
the transposed probabilities directly, pipelined one key-tile behind
the score stream. o_proj runs inside the attention pool scope (reusing
the score PSUM pool) so the last two heads' softmax finalizes hide
behind the first output tile's contraction.
"""

import math

import numpy as np

B, T, HID = 2, 2048, 2048
NH, NKV = 16, 8
NOPE, ROPE = 128, 64
HD = NOPE + ROPE  # 192
VD = 128
KV_RANK, Q_RANK = 512, 1536
EPS = 1e-6
THETA = 10000.0
NCORES = 8
TQ = B * T // NCORES  # 512 query tokens per core
P = 128
SCALE = 1.0 / math.sqrt(HD)

# Rope rows are stored "paired": each head's rotated rope halves (32+32
# rows) are stacked into one contiguous 64-row slot at base partition
# 64*(kvh%2), so the score-matmul lhsT(k)/rhs(q) base partitions match
# (PE only allows bases {0, 32, 64}).

_CACHE = {}


def _build_nc():
    import concourse.bass as bass  # noqa: F401
    import concourse.mybir as mybir
    from concourse import bacc
    from concourse.tile import TileContext

    F32 = mybir.dt.float32
    F32R = mybir.dt.float32r
    BF16 = mybir.dt.bfloat16
    F8 = mybir.dt.float8e4
    DR = mybir.MatmulPerfMode.DoubleRow
    AF = mybir.ActivationFunctionType
    ALU = mybir.AluOpType

    nc = bacc.Bacc(None, target_bir_lowering=False)

    xq_d = nc.dram_tensor("xq", [P, 16, TQ], BF16, kind="ExternalInput")
    qa_d = nc.dram_tensor("qa_w", [P, 12, 16, P], BF16, kind="ExternalInput")
    qb_d = nc.dram_tensor("qb_w", [P, 24, 12, P], BF16, kind="ExternalInput")
    kva_d = nc.dram_tensor("kva_w", [P, 16, 1024], BF16, kind="ExternalInput")
    kvb_d = nc.dram_tensor("kvb_w", [P, 4, 2048], BF16, kind="ExternalInput")
    o_d = nc.dram_tensor("o_w", [P, 4, 16, 512], BF16, kind="ExternalInput")
    cosq_d = nc.dram_tensor("cosq", [P, TQ], BF16, kind="ExternalInput")
    sinq_d = nc.dram_tensor("sinq", [P, TQ], BF16, kind="ExternalInput")
    onesb_d = nc.dram_tensor("ones_b", [P, P], BF16, kind="ExternalInput")
    # packed f32 tables: cols 0:128 all-ones (f32r lhsT for the softmax
    # denominator matmul), 128:132 kv ln weight * sqrt(rank), 132:134 eps
    tbl_d = nc.dram_tensor("tbl", [P, 134], F32R, kind="ExternalInput")
    qperm_d = nc.dram_tensor("qperm", [P, 4, P], BF16, kind="ExternalInput")
    # cross-core staging: this core's 512-key kv quarter + gathered full set
    kvl_p_d = nc.dram_tensor("kvl_p", [P, 4, TQ], BF16, kind="Internal")
    kprq_p_d = [nc.dram_tensor(f"kprq{j}_p", [64, 2, TQ], F8, kind="Internal")
                for j in range(4)]
    kvl_g_d = nc.dram_tensor("kvl_g", [4, P, 4, TQ], BF16, kind="Internal")
    kprg_d = [nc.dram_tensor(f"kprg{j}", [4, 64, 2, TQ], F8, kind="Internal")
              for j in range(4)]
    CC_GROUPS = [[0, 1, 2, 3], [4, 5, 6, 7]]
    out_d = nc.dram_tensor("out", [TQ, HID], F32, kind="ExternalOutput")

    with TileContext(nc) as tc:
        with tc.tile_pool(name="resident", bufs=1) as res:
            kv_latN = res.tile([P, 4, T], BF16, name="kv_latN")
            qnope = res.tile([P, NH, TQ], BF16, name="qnope")
            qrope = res.tile([64, 8, 2, TQ], F8, name="qrope")
            kpair = res.tile([64, 4, 2, T], F8, name="kpair")
            attn_sb = res.tile([P, NH, TQ], BF16, name="attn_sb")
            kvb_sb = res.tile([P, 4, 2048], BF16, name="kvb_sb")
            ones_sb = res.tile([P, P], BF16, name="ones_sb")
            tbl_sb = res.tile([P, 134], F32R, name="tbl_sb")

            # -- scat: rope-scatter sources, allocated at the TOP of SBUF
            # (side="right") so later phases' pools never overlap their
            # addresses and thus never wait on the background scatters.
            # -- pf1: kv_a/q inputs, freed before the attention phase.
            with (
                tc.tile_pool(name="scat", bufs=2, side="right") as scat,
                tc.tile_pool(name="pf1", bufs=1) as pf1,
            ):
                kva_sb = pf1.tile([P, 16, 1024], BF16, name="kva_sb")
                xq_sb = pf1.tile([P, 16, TQ], BF16, name="xq_sb")
                qperm_sb = pf1.tile([P, 4, P], BF16, name="qperm_sb")
                cosq_sb = scat.tile([P, TQ], BF16, name="cosq_sb", bufs=1)
                sinq_sb = scat.tile([P, TQ], BF16, name="sinq_sb", bufs=1)

                # ---- input streams ----
                # sync(SP) queue: xq chunks, then ONLY the collective-
                #   dependent gather lands + output stores (their sem waits
                #   hold the SP sequencer, which nothing else runs on)
                # scalar(ACT) queue: kva cols, qa/qb weight tiles, latent
                #   stage-out, odd kpr scatters, kvb, o_w
                for k0, k1 in ((0, 2), (2, 4), (4, 8), (8, 12), (12, 16)):
                    nc.sync.dma_start(
                        xq_sb[:, k0:k1, :],
                        xq_d[:, k0:k1, :],
                    )
                    nc.scalar.dma_start(
                        kva_sb[:, k0:k1, 0:512],
                        kva_d[:, k0:k1, 0:512],
                    )
                for c in range(2):
                    nc.scalar.dma_start(
                        kva_sb[:, 8 * c : 8 * c + 8, 512:1024],
                        kva_d[:, 8 * c : 8 * c + 8, 512:1024],
                    )
                nc.scalar.dma_start(kvb_sb[:], kvb_d[:, :, :])
                wsrc0 = res.tile([P, 64], BF16, name="wsrc0")
                nc.vector.memset(wsrc0[:], 0.0)
                nc.gpsimd.dma_start(ones_sb[:], onesb_d[:, :])
                nc.gpsimd.dma_start(tbl_sb[:], tbl_d[:, :])
                nc.gpsimd.dma_start(cosq_sb[:], cosq_d[:, :])
                nc.gpsimd.dma_start(sinq_sb[:], sinq_d[:, :])
                nc.gpsimd.dma_start(qperm_sb[:], qperm_d[:, :, :])

                # ---- PE warmup: burn the p-state ramp during initial DMA.
                with (
                    tc.tile_pool(name="wu", bufs=1) as wu,
                    tc.tile_pool(name="wups", bufs=1, space="PSUM") as wups,
                ):
                    wps = wups.tile([P, 64], F32, tag="wu")
                    for _ in range(72):
                        nc.tensor.matmul(
                            wps[0:64, :], wsrc0[:, :], wsrc0[:, :],
                            start=True, stop=True,
                        )

                # ------------- P1: kv_a for THIS core's 512 tokens ---------
                # Latent sweep first (k-outer so matmuls consume weight
                # chunks as they land), stage + AllGather A. Then the rope
                # sweep, rotation, paired scatter, AllGather B.
                with (
                    tc.tile_pool(name="p1l", bufs=1) as p1l,
                    tc.tile_pool(name="p1s", bufs=2) as p1s,
                    tc.tile_pool(name="p1ps", bufs=1, space="PSUM") as p1ps,
                    tc.tile_pool(name="p1ps1", bufs=1, space="PSUM") as p1ps1,
                ):
                    kvl_loc = p1l.tile([P, 4, TQ], BF16, name="kvl_loc")
                    raw1 = p1l.tile([P, 2, TQ], BF16, name="raw1")
                    raw2 = p1l.tile([P, 2, TQ], BF16, name="raw2")

                    # latent sweep: 4 live psum accumulators (m 0..3), k-outer
                    # so matmuls consume kva weight chunks as they land
                    lps = {}
                    for m in range(4):
                        lps[m] = p1ps.tile([P, TQ], F32, tag=f"kl{m}",
                                           name=f"kl{m}")
                    for k in range(16):
                        for m in range(4):
                            nc.tensor.matmul(
                                lps[m][:],
                                kva_sb[:, k, m * P : (m + 1) * P],
                                xq_sb[:, k, :],
                                start=(k == 0), stop=(k == 15),
                            )
                    ksumsq = p1ps1.tile([P, TQ], F32, tag="ksumsq")
                    for m in range(4):
                        nc.scalar.copy(kvl_loc[:, m, :], lps[m][:])
                    for m in range(4):
                        sq = p1s.tile([P, TQ], BF16, tag="ksq")
                        nc.vector.tensor_tensor(
                            sq[:], kvl_loc[:, m, :], kvl_loc[:, m, :], ALU.mult
                        )
                        nc.tensor.matmul(
                            ksumsq[:], ones_sb[:], sq[:],
                            start=(m == 0), stop=(m == 3),
                        )
                    ksqt = p1s.tile([P, TQ], F32, tag="ksqt", bufs=1)
                    nc.scalar.activation(
                        ksqt[:], ksumsq[:], AF.Sqrt, bias=tbl_sb[:, 132:133]
                    )
                    krs = p1s.tile([P, TQ], F32, tag="krs", bufs=1)
                    nc.vector.reciprocal(krs[:], ksqt[:])
                    for m in range(4):
                        nc.vector.scalar_tensor_tensor(
                            kvl_loc[:, m, :], kvl_loc[:, m, :],
                            tbl_sb[:, 128 + m : 129 + m], krs[:],
                            ALU.mult, ALU.mult,
                        )
                    nc.scalar.dma_start(kvl_p_d[:, :, :], kvl_loc[:, :, :])
                    nc.gpsimd.collective_compute(
                        "AllGather", ALU.bypass, CC_GROUPS,
                        ins=[kvl_p_d[:, :, :]], outs=[kvl_g_d[:, :, :, :]],
                    )

                    # rope sweep (kva cols 512:1024 -> m 4..7), reuses the
                    # latent psum tags
                    rps = {}
                    for m in range(4):
                        tg = f"kl{m}" if m < 2 else f"kr{m}"
                        rps[m] = p1ps.tile([P, TQ], F32, tag=tg,
                                           name=f"kr{m}")
                    for m in range(4):
                        for k in range(16):
                            nc.tensor.matmul(
                                rps[m][:],
                                kva_sb[:, k, 512 + m * P : 512 + (m + 1) * P],
                                xq_sb[:, k, :],
                                start=(k == 0), stop=(k == 15),
                            )
                    for m in range(4):
                        dst = raw1 if m < 2 else raw2
                        nc.scalar.copy(dst[:, m % 2, :], rps[m][:])

                    # rotate own keys (the rope tables for them are the
                    # query tables) and scatter straight to the DRAM part
                    ckb = cosq_sb[:, None, :].to_broadcast((P, 2, TQ))
                    skb = sinq_sb[:, None, :].to_broadcast((P, 2, TQ))
                    rt = scat.tile([P, 2, TQ], BF16, tag="rtmp", bufs=2)
                    r1 = scat.tile([P, 2, TQ], F8, tag="krot1", bufs=1)
                    rc1 = scat.tile([P, 2, TQ], BF16, tag="rc1", bufs=1)
                    nc.vector.tensor_tensor(rt[:], raw2[:], skb, ALU.mult)
                    nc.vector.tensor_tensor(rc1[:], raw1[:], ckb, ALU.mult)
                    nc.vector.tensor_tensor(r1[:], rc1[:], rt[:], ALU.subtract)
                    rt2 = scat.tile([P, 2, TQ], BF16, tag="rtmp", bufs=2)
                    rc2 = scat.tile([P, 2, TQ], BF16, tag="rc2", bufs=1)
                    r2 = scat.tile([P, 2, TQ], F8, tag="krot2", bufs=1)
                    nc.vector.tensor_tensor(rt2[:], raw1[:], skb, ALU.mult)
                    nc.vector.tensor_tensor(rc2[:], raw2[:], ckb, ALU.mult)
                    nc.vector.tensor_tensor(r2[:], rc2[:], rt2[:], ALU.add)
                    # head kvh -> slot kvh//2, base 64*(kvh%2)
                    for kvh in range(NKV):
                        t_, i = kvh // 4, kvh % 4
                        bb = 32 * (kvh % 2)
                        eng = nc.gpsimd if kvh % 2 == 0 else nc.scalar
                        kprd = kprq_p_d[kvh // 2]
                        eng.dma_start(
                            kprd[bb : bb + 32, 0, :],
                            r1[i * 32 : (i + 1) * 32, t_, :],
                        )
                        eng.dma_start(
                            kprd[bb : bb + 32, 1, :],
                            r2[i * 32 : (i + 1) * 32, t_, :],
                        )
                    for j in range(4):
                        nc.gpsimd.collective_compute(
                            "AllGather", ALU.bypass, CC_GROUPS,
                            ins=[kprq_p_d[j][:, :, :]], outs=[kprg_d[j][:, :, :, :]],
                        )
                    # land the gathered latents on the SWDGE queue: its ring
                    # is private, so the wait on the collective cannot poison
                    # the HWDGE rings the weight streams recycle through
                    for r in range(4):
                        ksl = slice(r * TQ, (r + 1) * TQ)
                        nc.gpsimd.dma_start(
                            kv_latN[:, :, ksl], kvl_g_d[r, :, :, :]
                        )

                # ------------- P2: q path -------------
                with (
                    tc.tile_pool(name="p2", bufs=1) as p2,
                    tc.tile_pool(name="p2w", bufs=4) as p2w,
                    tc.tile_pool(name="p2s", bufs=2) as p2s,
                    tc.tile_pool(name="p2ps", bufs=2, space="PSUM") as p2ps,
                    tc.tile_pool(name="p2ps1", bufs=1, space="PSUM") as p2ps1,
                ):
                    q_lat = p2.tile([P, 12, TQ], BF16, name="q_lat")
                    rs_q = p2.tile([P, TQ], F32, name="rs_q")

                    # q_a + rmsnorm statistics (ln*rs applied after q_b:
                    # ln is folded into the q_b rows on the host, rs is a
                    # per-token scale that commutes with q_b)
                    sumsq = p2ps1.tile([P, TQ], F32, tag="qsumsq")
                    for m in range(12):
                        wt = p2w.tile([P, 16, P], BF16, tag="w", name="wt")
                        nc.scalar.dma_start(wt[:], qa_d[:, m, :, :])
                        ps = p2ps.tile([P, TQ], F32, tag="mm", name="ps")
                        for k in range(16):
                            nc.tensor.matmul(
                                ps[:], wt[:, k, :], xq_sb[:, k, :],
                                start=(k == 0), stop=(k == 15),
                            )
                        nc.vector.tensor_copy(q_lat[:, m, :], ps[:])
                        sq = p2s.tile([P, TQ], BF16, tag="sq", name="sq", bufs=1)
                        if m < 2:
                            nc.vector.tensor_tensor(
                                sq[:], q_lat[:, m, :], q_lat[:, m, :], ALU.mult
                            )
                        else:
                            nc.scalar.square(sq[:], ps[:])
                        nc.tensor.matmul(
                            sumsq[:], ones_sb[:], sq[:],
                            start=(m == 0), stop=(m == 11),
                        )
                    sqt = p2s.tile([P, TQ], F32, tag="sqt", bufs=1)
                    nc.scalar.activation(
                        sqt[:], sumsq[:], AF.Sqrt, bias=tbl_sb[:, 133:134]
                    )
                    nc.vector.reciprocal(rs_q[:], sqt[:])

                    # q_b: nope heads to qnope, rope raw kept for rotation;
                    # the rs_q normalization rides on the PSUM->SBUF move.
                    # Host orders the rope halves in rounds of 8 heads:
                    # m=16,17: q1(h0-7), m=18,19: q2(h0-7),
                    # m=20,21: q1(h8-15), m=22,23: q2(h8-15).
                    qraws = {}
                    for m in list(range(16, 24)) + list(range(16)):
                        wt = p2w.tile([P, 16, P], BF16, tag="w")
                        nc.scalar.dma_start(wt[:, :12, :], qb_d[:, m, :, :])
                        ps = p2ps.tile([P, TQ], F32, tag="mm")
                        for k in range(12):
                            nc.tensor.matmul(
                                ps[:], wt[:, k, :], q_lat[:, k, :],
                                start=(k == 0), stop=(k == 11),
                            )
                        if m < 16:
                            dst = qnope[:, m, :]
                        else:
                            j = m - 16
                            half, idx = (j // 2) % 2, j % 2
                            if idx == 0:
                                qraws[half] = scat.tile(
                                    [P, 2, TQ], BF16, tag=f"qraw{half}",
                                    bufs=2, name=f"qraw{half}",
                                )
                            dst = qraws[half][:, idx, :]
                        nc.vector.tensor_tensor(dst, ps[:], rs_q[:], ALU.mult)
                        if m >= 16 and m % 4 == 3:
                            # rotate this round's 8 heads and scatter to the
                            # paired layout via the GPSIMD (SWDGE) queue
                            rnd = (m - 16) // 4
                            cb = cosq_sb[:, None, :].to_broadcast((P, 2, TQ))
                            sb_ = sinq_sb[:, None, :].to_broadcast((P, 2, TQ))
                            qr1, qr2 = qraws[0], qraws[1]
                            qt = scat.tile([P, 2, TQ], BF16, tag="qrtmp", bufs=2)
                            qc1 = scat.tile([P, 2, TQ], BF16, tag="qc1", bufs=2)
                            qo1 = scat.tile([P, 2, TQ], F8, tag="qrot1")
                            qo2 = scat.tile([P, 2, TQ], F8, tag="qrot2")
                            nc.vector.tensor_tensor(qt[:], qr2[:], sb_, ALU.mult)
                            nc.vector.tensor_tensor(qc1[:], qr1[:], cb, ALU.mult)
                            nc.vector.tensor_tensor(qo1[:], qc1[:], qt[:], ALU.subtract)
                            qt2 = scat.tile([P, 2, TQ], BF16, tag="qrtmp", bufs=2)
                            qc2 = scat.tile([P, 2, TQ], BF16, tag="qc2", bufs=2)
                            nc.vector.tensor_tensor(qt2[:], qr1[:], sb_, ALU.mult)
                            nc.vector.tensor_tensor(qc2[:], qr2[:], cb, ALU.mult)
                            nc.vector.tensor_tensor(qo2[:], qc2[:], qt2[:], ALU.add)
                            # head qh -> slot 2*(qh//4)+qh%2, base 64*((qh//2)%2)
                            for qh in range(8 * rnd, 8 * rnd + 8):
                                slot = 2 * (qh // 4) + qh % 2
                                bb = 32 * ((qh // 2) % 2)
                                src_r = (qh % 4) * 32
                                src_t = (qh % 8) // 4
                                nc.gpsimd.dma_start(
                                    qrope[bb : bb + 32, slot, 0, :],
                                    qo1[src_r : src_r + 32, src_t, :],
                                )
                                nc.gpsimd.dma_start(
                                    qrope[bb : bb + 32, slot, 1, :],
                                    qo2[src_r : src_r + 32, src_t, :],
                                )

                    # land the gathered rope-key quarters (SWDGE queue,
                    # after the qrope scatters in Pool order; each slot only
                    # waits its own collective)
                    for j in range(4):
                        for r in range(4):
                            ksl = slice(r * TQ, (r + 1) * TQ)
                            nc.gpsimd.dma_start(
                                kpair[:, j, :, ksl], kprg_d[j][r, :, :, :]
                            )

            # ------------- P3 + P4 (pf1 SBUF freed) -----------------------
            with tc.tile_pool(name="oww", bufs=4) as oww:
                ow_tiles = {}

                def ow_load(n, eng):
                    ow = oww.tile([P, 16, 512], BF16, tag="ow")
                    eng.dma_start(ow[:], o_d[:, n, :, :])
                    ow_tiles[n] = ow

                with (
                    tc.tile_pool(name="p3", bufs=2) as p3,
                    tc.tile_pool(name="p3q", bufs=2) as p3q,
                    tc.tile_pool(name="p3p", bufs=4) as p3p,
                    tc.tile_pool(name="scps", bufs=3, space="PSUM") as scps,
                    tc.tile_pool(name="atps", bufs=2, space="PSUM") as atps,
                ):
                    pending = []

                    def finalize(item):
                        dsum, at, qh = item
                        dnp = scps.tile([P, 2, TQ], F32, tag="sc", name="dnp")
                        dn = dnp[:, 0, :]
                        nc.tensor.matmul(
                            dn, ones_sb[:], dsum[:], start=True, stop=True
                        )
                        rec = p3q.tile([P, TQ], F32, tag="rec")
                        nc.vector.reciprocal(rec[:], dn)
                        nc.vector.tensor_tensor(
                            attn_sb[:, qh, :], at[:], rec[:], ALU.mult
                        )

                    # kv_b preamble for ONE head-pair: needs only the
                    # gathered LATENTS. Pipelined 2 stages ahead of the
                    # attention loop (double-buffered), so preambles hp0+hp1
                    # bridge the window between the latent gather and the
                    # rope-key gather while hp2/hp3 hide inside attention.
                    def preamble(hp):
                        kvh0 = 2 * hp
                        knp = p3.tile([P, 2, T], BF16, tag="knp", name="knp")
                        vp = p3.tile([P, 16, 256], BF16, tag="vp", name="vp")
                        for h2 in range(2):
                            wsl = slice((kvh0 + h2) * NOPE, (kvh0 + h2 + 1) * NOPE)
                            for n4 in range(4):
                                ksl = slice(n4 * 512, (n4 + 1) * 512)
                                psp = scps.tile([P, 2, TQ], F32, tag="sc",
                                                name="psp")
                                for r in range(4):
                                    nc.tensor.matmul(
                                        psp[:, 0, :], kvb_sb[:, r, wsl],
                                        kv_latN[:, r, ksl],
                                        start=(r == 0), stop=(r == 3),
                                    )
                                nc.vector.tensor_copy(knp[:, h2, ksl],
                                                      psp[:, 0, :])
                        vsl = slice(NKV * NOPE + kvh0 * VD, NKV * NOPE + (kvh0 + 2) * VD)
                        for kt in range(16):
                            psp = scps.tile([P, 2, TQ], F32, tag="sc",
                                            name="psp2")
                            for r in range(4):
                                nc.tensor.matmul(
                                    psp[:, 0, :256],
                                    kv_latN[:, r, kt * P : (kt + 1) * P],
                                    kvb_sb[:, r, vsl],
                                    start=(r == 0), stop=(r == 3),
                                )
                            if (hp == 1 and kt >= 12) or hp >= 2:
                                nc.vector.tensor_copy(vp[:, kt, :],
                                                      psp[:, 0, :256])
                            else:
                                nc.scalar.copy(vp[:, kt, :], psp[:, 0, :256])
                        if hp > 0:
                            # o_proj weight prefetch on the ACT queue
                            ow_load(hp - 1, nc.scalar)
                        if hp == 3:
                            ow_load(3, nc.scalar)
                        return knp, vp

                    kvp = {0: preamble(0), 1: preamble(1)}
                    for hp in range(4):  # kv-head pairs
                        kvh0 = 2 * hp
                        knp, vp = kvp.pop(hp)

                        for j4 in range(4):
                            qh = 4 * hp + j4
                            kvh = qh // 2
                            h2 = kvh - kvh0
                            b = 32 * (kvh % 2)
                            slot = 2 * (qh // 4) + qh % 2
                            dsum = p3q.tile([P, TQ], BF16, tag="dsum")
                            dsum2 = p3q.tile([P, TQ], BF16, tag="dsum2")
                            at = atps.tile([P, TQ], F32, tag="at")
                            pts = {}
                            for kp in range(8):  # key-tile pairs, one exp each
                                sc = scps.tile([P, 2, TQ], F32, tag="sc")
                                for half in range(2):
                                    kt = 2 * kp + half
                                    nc.tensor.matmul(
                                        sc[:, half, :],
                                        knp[:, h2, kt * P : (kt + 1) * P],
                                        qnope[:, qh, :],
                                        start=True, stop=False,
                                    )
                                    nc.tensor.matmul(
                                        sc[:, half, :],
                                        kpair[b : b + 32, kvh // 2, :, kt * P : (kt + 1) * P],
                                        qrope[b : b + 32, slot, :, :],
                                        start=False, stop=True,
                                        perf_mode=DR,
                                    )
                                pt = p3p.tile([P, 2, TQ], BF16, tag="pt")
                                nc.scalar.activation(
                                    pt[:], sc[:], AF.Exp, scale=float(SCALE)
                                )
                                pts[kp] = pt
                                d_ = dsum if kp % 2 == 0 else dsum2
                                if kp < 2:
                                    nc.vector.tensor_copy(d_[:], pt[:, 0, :])
                                    nc.vector.tensor_tensor(
                                        d_[:], d_[:], pt[:, 1, :], ALU.add
                                    )
                                else:
                                    nc.vector.tensor_tensor(
                                        d_[:], d_[:], pt[:, 0, :], ALU.add
                                    )
                                    nc.vector.tensor_tensor(
                                        d_[:], d_[:], pt[:, 1, :], ALU.add
                                    )
                                if kp > 1:  # PV two pairs behind scores
                                    for half in range(2):
                                        kt = 2 * (kp - 2) + half
                                        nc.tensor.matmul(
                                            at[:],
                                            vp[:, kt, h2 * VD : (h2 + 1) * VD],
                                            pts[kp - 2][:, half, :],
                                            start=(kt == 0), stop=False,
                                        )
                                    del pts[kp - 2]
                            for half in range(2):
                                kt = 12 + half
                                nc.tensor.matmul(
                                    at[:],
                                    vp[:, kt, h2 * VD : (h2 + 1) * VD],
                                    pts[6][:, half, :],
                                    start=False, stop=False,
                                )
                            # the previous head's finalize slots here, giving
                            # the last exp time to land before its PV
                            if pending:
                                finalize(pending.pop(0))
                            for half in range(2):
                                kt = 14 + half
                                nc.tensor.matmul(
                                    at[:],
                                    vp[:, kt, h2 * VD : (h2 + 1) * VD],
                                    pts[7][:, half, :],
                                    start=False, stop=(half == 1),
                                )
                            del pts[6], pts[7]
                            nc.vector.tensor_tensor(
                                dsum[:], dsum[:], dsum2[:], ALU.add
                            )
                            pending.append((dsum, at, qh))
                        if hp + 2 < 4:
                            kvp[hp + 2] = preamble(hp + 2)
                    # ---- P4: o_proj, inside the attention pools so the
                    # last two heads' finalizes hide behind the first chain
                    for n in range(4):
                        ow = ow_tiles[n]
                        for mt in range(4):
                            last = n == 3 and mt == 3
                            if not last:
                                psp = scps.tile([P, 2, TQ], F32, tag="sc",
                                                name="ops")
                                ps = psp[:, 0, :]
                                for h in range(NH):
                                    if h == 14 and pending:
                                        while pending:
                                            finalize(pending.pop(0))
                                    nc.tensor.matmul(
                                        ps,
                                        attn_sb[:, h, mt * P : (mt + 1) * P],
                                        ow[:, h, :],
                                        start=(h == 0), stop=(h == 15),
                                    )
                                st = p3q.tile([P, TQ], F32, tag="st",
                                              name="st")
                                nc.scalar.copy(st[:], ps)
                                nc.sync.dma_start(
                                    out_d[mt * P : (mt + 1) * P,
                                          n * 512 : (n + 1) * 512],
                                    st[:],
                                )
                                continue
                            # split the final tile so the first half's copy
                            # and store overlap the second half's matmuls
                            for c0, c1 in ((0, 384), (384, 512)):
                                w = c1 - c0
                                psp = scps.tile([P, 2, TQ], F32, tag="sc",
                                                name="ops2")
                                for h in range(NH):
                                    nc.tensor.matmul(
                                        psp[:, 0, :w],
                                        attn_sb[:, h, mt * P : (mt + 1) * P],
                                        ow[:, h, c0:c1],
                                        start=(h == 0), stop=(h == 15),
                                    )
                                st = p3q.tile([P, TQ], F32, tag="st",
                                              name="st2")
                                nc.scalar.copy(st[:, :w], psp[:, 0, :w])
                                nc.sync.dma_start(
                                    out_d[mt * P : (mt + 1) * P,
                                          n * 512 + c0 : n * 512 + c1],
                                    st[:, :w],
                                )

    nc.finalize()
    return nc


def _host_prep(inputs):
    import ml_dtypes

    BF = ml_dtypes.bfloat16

    def bf(a):
        return np.ascontiguousarray(np.asarray(a, dtype=np.float32).astype(BF))

    x = np.asarray(inputs["hidden_states"], dtype=np.float32)

    qa_w = np.asarray(inputs["q_a_w"], np.float32)  # [HID, Q_RANK]
    qa_t = bf(qa_w.reshape(16, P, 12, P).transpose(1, 2, 0, 3))

    # fold the q rmsnorm weight (and the sqrt(rank) factor of the mean)
    # into the q_b rows; the per-token rsqrt is applied after q_b on-device
    lnq = (np.asarray(inputs["q_a_ln_w"], np.float64) * math.sqrt(Q_RANK)).astype(
        np.float32
    )
    qb = np.asarray(inputs["q_b_w"], np.float32) * lnq[:, None]
    qb = qb.reshape(Q_RANK, NH, HD)
    nope_cols = qb[:, :, :NOPE].reshape(Q_RANK, NH * NOPE)
    rope1 = qb[:, :, NOPE : NOPE + 32].reshape(Q_RANK, NH * 32)
    rope2 = qb[:, :, NOPE + 32 :].reshape(Q_RANK, NH * 32)
    # rope halves in rounds of 8 heads: q1(h0-7), q2(h0-7), q1(h8-15), q2(h8-15)
    qb_cols = np.concatenate(
        [nope_cols, rope1[:, :256], rope2[:, :256], rope1[:, 256:], rope2[:, 256:]],
        axis=1,
    )  # [1536, 3072]
    qb_t = bf(qb_cols.reshape(12, P, 24, P).transpose(1, 2, 0, 3))

    kva = np.asarray(inputs["kv_a_w"], np.float32)
    lat = kva[:, :KV_RANK]
    krope = kva[:, KV_RANK:].reshape(HID, NKV, ROPE)
    kr1 = krope[:, :, :32].reshape(HID, NKV * 32)
    kr2 = krope[:, :, 32:].reshape(HID, NKV * 32)
    kva_cols = np.concatenate([lat, kr1, kr2], axis=1)  # [2048, 1024]
    kva_t = bf(kva_cols.reshape(16, P, 1024).transpose(1, 0, 2))

    kvb = np.asarray(inputs["kv_b_w"], np.float32).reshape(KV_RANK, NKV, NOPE + VD)
    knope_cols = kvb[:, :, :NOPE].reshape(KV_RANK, NKV * NOPE)
    v_cols = kvb[:, :, NOPE:].reshape(KV_RANK, NKV * VD)
    kvb_cols = np.concatenate([knope_cols, v_cols], axis=1)  # [512, 2048]
    kvb_t = bf(kvb_cols.reshape(4, P, 2048).transpose(1, 0, 2))

    o_w = np.asarray(inputs["o_w"], np.float32)  # [NH*VD, HID]
    o_t = bf(o_w.reshape(16, P, 4, 512).transpose(1, 2, 0, 3))

    lnkv = (
        (np.asarray(inputs["kv_a_ln_w"], np.float64) * math.sqrt(KV_RANK))
        .astype(np.float32)
        .reshape(4, P)
        .T
    )
    tbl = np.empty((P, 134), np.float32)
    tbl[:, 0:128] = 1.0
    tbl[:, 128:132] = lnkv
    tbl[:, 132] = EPS * KV_RANK
    tbl[:, 133] = EPS * Q_RANK

    inv_freq = 1.0 / (THETA ** (np.arange(0, ROPE, 2, dtype=np.float32) / ROPE))
    t = np.arange(T, dtype=np.float32)
    freqs = np.outer(t, inv_freq).astype(np.float32)
    cosk = np.tile(np.cos(freqs).T, (4, 1))  # [128, T]
    sink = np.tile(np.sin(freqs).T, (4, 1))
    cosk_b, sink_b = bf(cosk), bf(sink)
    ones_b = np.ones((P, P), BF)
    qperm = np.zeros((4, P, P), np.float32)
    for q in list(range(0, 32)) + list(range(64, 96)):
        qperm[0, q, q] = 1.0
        qperm[1, q, q + 32] = 1.0
    for q in list(range(32, 64)) + list(range(96, 128)):
        qperm[2, q, q - 32] = 1.0
        qperm[3, q, q] = 1.0
    qperm_t = bf(qperm.transpose(1, 0, 2))

    in_maps = []
    for c in range(NCORES):
        b, qc = c // 4, c % 4
        xTb = x[b].T  # [HID, T]
        qoff = qc * TQ
        xq_t = bf(xTb[:, qoff : qoff + TQ].reshape(16, P, TQ).transpose(1, 0, 2))
        in_maps.append(
            {
                "xq": xq_t,
                "qa_w": qa_t,
                "qb_w": qb_t,
                "kva_w": kva_t,
                "kvb_w": kvb_t,
                "o_w": o_t,
                "cosq": np.ascontiguousarray(cosk_b[:, qoff : qoff + TQ]),
                "sinq": np.ascontiguousarray(sink_b[:, qoff : qoff + TQ]),
                "ones_b": ones_b,
                "tbl": tbl,
                "qperm": qperm_t,
            }
        )
    return in_maps


def get_nc():
    if "nc" not in _CACHE:
        _CACHE["nc"] = _build_nc()
    return _CACHE["nc"]


def kernel(**inputs) -> np.ndarray:
    from concourse.bass_utils import run_bass_kernel_spmd

    nc = get_nc()
    in_maps = _host_prep(inputs)
    res = run_bass_kernel_spmd(nc, in_maps, core_ids=list(range(NCORES)))
    _CACHE["last_result"] = res
    outs = [res.results[c]["out"] for c in range(NCORES)]
    full = np.stack(
        [np.concatenate([outs[b * 4 + qc] for qc in range(4)], axis=0) for b in range(B)]
    )
    return full.astype(np.float32)
